# revision 52
# baseline (speedup 1.0000x reference)
"""Trainium2 Bass kernel for LocalDenseSynthesizerAttention.

Data-parallel over batch B=8 -> 8 cores, one batch each. The axon tunnel
(~45MB/s, effectively half-duplex) dominates, so the design minimizes wire
bytes (~21.8MB/call vs 43.4MB for the fp8/bf16 baseline; ~2.1x faster):
  - q shipped as packed 5-bit codes (8 codes -> 5 bytes) with per-row
    bf16-exact scales: the attention path is heavily damped (softmax over a
    45-wide window of tiny logits), so 5 bits cost only ~2e-3 of rel-err
  - v shipped int8 with per-64-block fp8 scales (pre-scaled x64 into fp8's
    normal range); v ships ONCE as a single zero-padded tensor shared by all
    chunk launches (window is local, pad=22), no per-chunk halo duplication
  - output quantized ON DEVICE to packed 7-bit codes (8 codes -> 7 bytes)
    with per-row f32 scales, unpacked/dequantized on host
  - all pack/unpack on device uses exact fp32 integer arithmetic (u8->f32
    convert, power-of-2 scaled floor via the +1.5*2^23 magic-add round), so
    device decode is bit-identical to the host's integer packing and
    independent of any convert-instruction rounding mode
  - projection weights shipped f32 ONCE (content-compared per call, reuses
    device-resident copies), AllGathered from 8-way shards on device, stored
    pre-transposed [128, KF, N] fp16 for the compute launches
  - device compute in fp16 (PE supports fp16 matmul): the extra mantissa
    bits vs bf16 pay for the int8/int5 wire budget; total measured rel-err
    1.63e-2 (gate 2e-2), bit-reproducible and matching the numpy simulation
  - codes+scales packed into one buffer per tensor (few device_puts; each
    put has a large fixed cost), puts dispatched from a small thread pool
  - compute split into 4 sequence chunks, one 8-core launch per chunk, so
    host quantization, uploads, exec, downloads and host dequant pipeline

The local window C=45 weighted sum is computed as banded matmuls: the banded
matrix B[s,t'] = attn[t0+t',h,s-t'] is an affine strided view of a zero-padded
attn tensor in DRAM, loaded matmul-ready via XBAR transpose-DMA.

Env knobs (defaults are the shipped config): KQBITS=5|6|8, KVBLK=1|0,
KOUT7=1|0, KCHUNKS=csv of chunk lengths, KTIMING=1 for per-phase timings.
NOTE: KQBITS=6 with KOUT7=1 miscompiles (a scheduling hazard corrupts the
last tile); both features are individually fine and the default q5+out7
combination is validated end-to-end.

Self-contained: hardcodes shapes from the problem spec.
"""
import sys
sys.path.insert(0, '/opt/trn_rl_repo')
import numpy as np

import concourse.bass as bass
import concourse.mybir as mybir
import concourse.tile as tile
from concourse import bacc
from concourse import masks

T, F = 2048, 512
H, C, DK = 8, 45, 64
HC = H * C          # 360
W = 128             # padded attn width per head (covers s-t' in [-63,127])
S = 64              # t' band-block size
PADV = 22           # (C-1)//2
KF = F // 128       # 4 contraction chunks
B = 8               # total batches / cores
FSH = F // B        # 64 weight-shard rows per core

VH = 64             # v halo rows each side (>= PADV, keeps tiles 128-aligned)
VOFF = VH - PADV    # chunk-vpad[r] = v_logical[r + VOFF]
CHUNKS = [(0, 512), (512, 512), (1024, 512), (1536, 512)]
import os as _os
if _os.environ.get("KCHUNKS"):
    _ls = [int(x) for x in _os.environ["KCHUNKS"].split(",")]
    assert sum(_ls) == T
    CHUNKS = []
    _c = 0
    for _l in _ls:
        CHUNKS.append((_c, _l))
        _c += _l

F16 = mybir.dt.float16
F32 = mybir.dt.float32
I8 = mybir.dt.int8
U8 = mybir.dt.uint8
FP8 = mybir.dt.float8e4
QD = 126.0          # int8 quant denominator (126 leaves headroom for the
                    # bf16 round-down of the scale: 126*1.002 < 126.5)
QD6 = 30.0          # 6-bit quant denominator (codes in [-30, 30])
QD5 = 15.0          # 5-bit quant denominator (codes in [-15, 15])
VDN = 118.0         # v block-quant denominator (fp8 scale round-down can
                    # inflate codes by up to 6.25%: 118*1.0625 < 127)
MAGIC = 12582912.0  # 1.5 * 2^23: fp32 add rounds the value to nearest int
QBITS = int(_os.environ.get("KQBITS", "5"))   # q wire precision: 5, 6, or 8
QROW = {5: 320, 6: 384, 8: 512}[QBITS]        # q bytes per row on the wire
VBLK = _os.environ.get("KVBLK", "1") == "1"   # v block-64 quant (else row)
NVS = 8 if VBLK else 4   # v scale bytes per row (8 fp8 / one f32)
OUT7 = _os.environ.get("KOUT7", "1") == "1"   # output packed 7-bit
# v ships as several tensors split at these rows (each 64 mod 128, so no
# halo-offset tile straddles a boundary); a chunk launch gates only on the
# parts it reads, so early chunks' downloads overlap later uploads
VBOUNDS = [0, 576, 1088, 1600, T]
OROW = 448 if OUT7 else 512                   # output bytes per row
ODN = 62.0 if OUT7 else QD                    # output quant denominator

_CACHE = {}


# The build functions are compiled from a synthetic filename so the
# source-location debug info embedded in the BIR (and thus the NEFF
# cache key) does not depend on where this file lives.
_BUILD_SRC = r'''
def _build_w():
    """Weights launch (first call only): AllGather 8-way f32 weight shards,
    convert to fp16 in the matmul-ready [128, KF, N] layout, store to
    device-resident DRAM outputs."""
    nc = bacc.Bacc("TRN2", target_bir_lowering=False, debug=False,
                   num_devices=B, disable_frame_to_traceback=True)
    w1s = nc.dram_tensor("w1s", (FSH, F), F32, kind="ExternalInput")
    w2s = nc.dram_tensor("w2s", (FSH, HC), F32, kind="ExternalInput")
    w3s = nc.dram_tensor("w3s", (FSH, F), F32, kind="ExternalInput")
    wos = nc.dram_tensor("wos", (FSH, F), F32, kind="ExternalInput")
    w1f = nc.dram_tensor("w1f", (128, KF * F), F16, kind="ExternalOutput")
    w2f = nc.dram_tensor("w2f", (128, KF * HC), F16, kind="ExternalOutput")
    w3f = nc.dram_tensor("w3f", (128, KF * F), F16, kind="ExternalOutput")
    wof = nc.dram_tensor("wof", (128, KF * F), F16, kind="ExternalOutput")
    groups = [list(range(B))]
    with tile.TileContext(nc) as tc:
        with tc.tile_pool(name="dram", bufs=1, space="DRAM") as dp, \
             tc.tile_pool(name="sb", bufs=2) as sp:
            for idx, (shard, out, n) in enumerate(
                    ((w1s, w1f, F), (w2s, w2f, HC),
                     (w3s, w3f, F), (wos, wof, F))):
                stage = dp.tile([FSH, n], F32, name=f"st{idx}")
                full = dp.tile([F, n], F32, name=f"fu{idx}")
                # collectives cannot read IO tensors: stage shards first
                nc.sync.dma_start(stage[:, :], shard[:, :])
                nc.gpsimd.collective_compute(
                    "AllGather", mybir.AluOpType.bypass, groups,
                    [stage[:, :]], [full[:, :]])
                sb32 = sp.tile([128, KF, n], F32, tag=f"sb32_{n}")
                nc.sync.dma_start(
                    sb32[:], full[:, :].rearrange("(ko p) n -> p ko n", p=128))
                sb16 = sp.tile([128, KF, n], F16, tag=f"sb16_{n}")
                nc.scalar.copy(sb16[:], sb32[:])
                nc.sync.dma_start(
                    out[:, :].rearrange("p (ko n) -> p ko n", ko=KF), sb16[:])
    nc.compile()
    return nc


def _build_k(TC, C0):
    """Compute launch for one sequence chunk of TC rows starting at C0."""
    TCV = TC + 2 * VH           # v rows incl halo
    NT = TC // 128              # t-tiles in the chunk
    NTV = TCV // 128            # v tiles incl halo
    NB = TC // S                # band blocks
    nc = bacc.Bacc("TRN2", target_bir_lowering=False, debug=False,
                   num_devices=B, disable_frame_to_traceback=True)
    # qb = q rows (TC x QROW bytes: int8 codes, or 5/6-bit codes packed in
    # byte planes) then TC f32 row scales (raw bytes);
    # v parts (shared by all chunk launches), each codes-then-scales; a
    # chunk declares only the parts it reads, so its launch is not gated on
    # the other parts' uploads. Logical padded row r maps to v row r - VH;
    # the VH-row sequence-edge pads are zero-filled on device, not shipped.
    qb = nc.dram_tensor("qb", (TC * QROW + 4 * TC,), I8, kind="ExternalInput")
    vparts = []            # (tensor, row0, nrows) for the declared parts
    r_lo = max(C0 - VH, 0)                    # first v row this chunk reads
    r_hi = min(C0 + TC + VH, T)               # one past the last
    for pi in range(len(VBOUNDS) - 1):
        b0, b1 = VBOUNDS[pi], VBOUNDS[pi + 1]
        if r_lo < b1 and r_hi > b0:
            nn = b1 - b0
            vt = nc.dram_tensor(f"vp{pi}", (nn * F + NVS * nn,), I8,
                                kind="ExternalInput")
            vparts.append((vt, b0, nn))
    w1f = nc.dram_tensor("w1f", (128, KF * F), F16, kind="ExternalInput")
    w2f = nc.dram_tensor("w2f", (128, KF * HC), F16, kind="ExternalInput")
    w3f = nc.dram_tensor("w3f", (128, KF * F), F16, kind="ExternalInput")
    wof = nc.dram_tensor("wof", (128, KF * F), F16, kind="ExternalInput")
    oq = nc.dram_tensor("oq", (TC, OROW), I8, kind="ExternalOutput")
    os_ = nc.dram_tensor("os", (TC, 1), F32, kind="ExternalOutput")

    with tile.TileContext(nc) as tc:
        with tc.tile_pool(name="wpool", bufs=1) as wp, \
             tc.tile_pool(name="inpool", bufs=1) as inp, \
             tc.tile_pool(name="persist", bufs=1) as pers, \
             tc.tile_pool(name="work", bufs=2) as wk, \
             tc.tile_pool(name="band", bufs=4) as bp, \
             tc.tile_pool(name="psmain", bufs=2, space="PSUM") as psm, \
             tc.tile_pool(name="psband", bufs=4, space="PSUM") as psb, \
             tc.tile_pool(name="pstp", bufs=2, space="PSUM") as ptp, \
             tc.tile_pool(name="drampool", bufs=1, space="DRAM") as dp:

            # ---- weights to SBUF, [128, KF, n] fp16 (partition = contraction)
            w1_t = wp.tile([128, KF, F], F16, tag="w1")
            nc.sync.dma_start(
                w1_t[:], w1f[:, :].rearrange("p (ko n) -> p ko n", ko=KF))
            w2_t = wp.tile([128, KF, HC], F16, tag="w2")
            nc.sync.dma_start(
                w2_t[:], w2f[:, :].rearrange("p (ko n) -> p ko n", ko=KF))
            w3_t = wp.tile([128, KF, F], F16, tag="w3")
            nc.sync.dma_start(
                w3_t[:], w3f[:, :].rearrange("p (ko n) -> p ko n", ko=KF))
            wo_t = wp.tile([128, KF, F], F16, tag="wo")
            nc.sync.dma_start(
                wo_t[:], wof[:, :].rearrange("p (ko n) -> p ko n", ko=KF))

            ident = pers.tile([128, 128], F16, tag="ident")
            masks.make_identity(nc, ident[:])

            # ---- dequantize q and v (t-major int8 -> fp16), PE-transpose to
            # f-major [128 f, KF, t]
            qT = inp.tile([128, KF, TC], F16, tag="qT")
            vT = inp.tile([128, KF, TCV], F16, tag="vT")

            def _scale_ap(src_t, nrow, nbytes, tt):
                return src_t[nrow * nbytes + 4 * tt * 128:
                             nrow * nbytes + 4 * (tt + 1) * 128] \
                    .bitcast(F32).rearrange("(p n) -> p n", n=1)

            def _transpose_in(dst, dq, tt):
                for fo in range(KF):
                    pst = ptp.tile([128, 128], F16, tag="tp")
                    nc.tensor.transpose(
                        pst[:], dq[:, fo * 128:(fo + 1) * 128], ident[:])
                    nc.scalar.copy(dst[:, fo, tt * 128:(tt + 1) * 128],
                                   pst[:])

            A = mybir.AluOpType
            ts = nc.vector.tensor_scalar

            def _floor(out, in_, inv):
                # floor(in_/d) for small nonneg d-multiples-of-1/d: exact
                # round(in_*inv - 63/128) via the fp32 magic add
                ts(out, in_, inv, -0.4921875, A.mult, A.add)
                ts(out, out, MAGIC, -MAGIC, A.add, A.add)

            # ---- q tiles: unpack (if packed), dequant, transpose
            for tt in range(NT):
                sq = wk.tile([128, 1], F32, tag="sq")
                nc.sync.dma_start(sq[:], _scale_ap(qb, TC, QROW, tt))
                dq = wk.tile([128, F], F16, tag="dq")
                if QBITS == 8:
                    i8 = wk.tile([128, F], I8, tag="i8")
                    src = qb[tt * 128 * F:(tt + 1) * 128 * F]
                    nc.sync.dma_start(
                        i8[:], src.rearrange("(p n) -> p n", n=F))
                    nc.scalar.activation(dq[:], i8[:],
                                         mybir.ActivationFunctionType.Copy,
                                         scale=sq[:, :])
                    _transpose_in(qT, dq, tt)
                    continue
                # packed path: bytes hold a low field (6 or 5 bits) plus
                # high bits that assemble the remaining codes. All
                # arithmetic is exact fp32 on small integers.
                pk = wk.tile([128, QROW], U8, tag="pk")
                src = qb[tt * 128 * QROW:(tt + 1) * 128 * QROW].bitcast(U8)
                nc.sync.dma_start(
                    pk[:], src.rearrange("(p n) -> p n", n=QROW))
                pf = wk.tile([128, QROW], F32, tag="pf")
                nc.scalar.copy(pf[:], pk[:])
                hi = wk.tile([128, QROW], F32, tag="hi")
                lo = wk.tile([128, QROW], F32, tag="lo")
                ct = wk.tile([128, F], F32, tag="ct")
                if QBITS == 6:
                    # planes P_j (128 cols): low 6 = code of f=4k+j; high 2
                    # = bits [2j,2j+2) of the f=4k+3 code
                    _floor(hi[:], pf[:], 1.0 / 64)
                    ts(lo[:], hi[:], -64.0, None, A.mult)
                    nc.vector.tensor_add(out=lo[:], in0=lo[:], in1=pf[:])
                    c4 = ct[:].rearrange("p (k j) -> p k j", j=4)
                    for j in range(3):
                        nc.vector.tensor_copy(
                            out=c4[:, :, j], in_=lo[:, j * 128:(j + 1) * 128])
                    t1 = wk.tile([128, 128], F32, tag="t1")
                    ts(t1[:], hi[:, 128:256], 4.0, None, A.mult)
                    nc.vector.tensor_add(out=t1[:], in0=t1[:],
                                         in1=hi[:, 0:128])
                    t2 = wk.tile([128, 128], F32, tag="t2")
                    ts(t2[:], hi[:, 256:384], 16.0, None, A.mult)
                    nc.vector.tensor_add(out=c4[:, :, 3], in0=t1[:],
                                         in1=t2[:])
                    qbias = -32.0
                else:
                    # QBITS == 5: planes P_j (j<5, 64 cols): low 5 = code of
                    # f=8k+j; high 3 = bits [3j,3j+3) of G = c5|c6<<5|c7<<10
                    _floor(hi[:], pf[:], 1.0 / 32)
                    ts(lo[:], hi[:], -32.0, None, A.mult)
                    nc.vector.tensor_add(out=lo[:], in0=lo[:], in1=pf[:])
                    c8 = ct[:].rearrange("p (k j) -> p k j", j=8)
                    for j in range(5):
                        nc.vector.tensor_copy(
                            out=c8[:, :, j], in_=lo[:, j * 64:(j + 1) * 64])
                    g = wk.tile([128, 64], F32, tag="g")
                    t1 = wk.tile([128, 64], F32, tag="t1")
                    ts(g[:], hi[:, 64:128], 8.0, None, A.mult)
                    nc.vector.tensor_add(out=g[:], in0=g[:], in1=hi[:, 0:64])
                    for j, w in ((2, 64.0), (3, 512.0), (4, 4096.0)):
                        ts(t1[:], hi[:, j * 64:(j + 1) * 64], w, None, A.mult)
                        nc.vector.tensor_add(out=g[:], in0=g[:], in1=t1[:])
                    f1 = wk.tile([128, 64], F32, tag="f1")
                    _floor(f1[:], g[:], 1.0 / 32)
                    ts(t1[:], f1[:], -32.0, None, A.mult)
                    nc.vector.tensor_add(out=c8[:, :, 5], in0=t1[:],
                                         in1=g[:])
                    f2 = wk.tile([128, 64], F32, tag="f2")
                    _floor(f2[:], f1[:], 1.0 / 32)
                    ts(t1[:], f2[:], -32.0, None, A.mult)
                    nc.vector.tensor_add(out=c8[:, :, 6], in0=t1[:],
                                         in1=f1[:])
                    nc.vector.tensor_copy(out=c8[:, :, 7], in_=f2[:])
                    qbias = -16.0
                # dequant: (c + qbias) * s = c*s + qbias*s
                nbias = wk.tile([128, 1], F32, tag="nbias")
                ts(nbias[:], sq[:], qbias, None, A.mult)
                nc.scalar.activation(dq[:], ct[:],
                                     mybir.ActivationFunctionType.Identity,
                                     bias=nbias[:, :], scale=sq[:, :])
                _transpose_in(qT, dq, tt)

            # ---- v tiles: int8 dequant, transpose. Tile tt covers padded
            # rows [C0+128tt, C0+128(tt+1)) = vb rows shifted by -VH; tiles
            # overlapping the sequence edge are zero-filled then partially
            # loaded.
            for tt in range(NTV):
                p0 = C0 + tt * 128          # first padded row of the tile
                a = max(p0, VH) - p0        # first valid partition
                b = min(p0 + 128, VH + T) - p0   # one past last valid
                r0 = p0 - VH + a            # first v row
                nrow = b - a
                vt, vr0, vnr = next((t, o, nn) for t, o, nn in vparts
                                    if o <= r0 < o + nn)
                assert r0 + nrow <= vr0 + vnr   # aligned split: no straddle
                rr = r0 - vr0
                i8 = wk.tile([128, F], I8, tag="i8")
                if nrow < 128:
                    nc.any.memzero(i8[:])
                src = vt[rr * F:(rr + nrow) * F]
                nc.sync.dma_start(
                    i8[a:b, :], src.rearrange("(p n) -> p n", n=F))
                dq = wk.tile([128, F], F16, tag="dq")
                if VBLK:
                    # 8 fp8 block scales per row, pre-multiplied by 64 on
                    # the host so they sit in fp8's normal range
                    s8 = wk.tile([128, 8], FP8, tag="s8")
                    if nrow < 128:
                        # fp8 1.0 in the pad rows (codes there are 0; any
                        # finite scale works, garbage could decode to NaN)
                        nc.any.memset(s8[:], 1.0)
                    ssrc = vt[vnr * F + 8 * rr:vnr * F + 8 * (rr + nrow)] \
                        .bitcast(FP8).rearrange("(p n) -> p n", n=8)
                    nc.sync.dma_start(s8[a:b, :], ssrc)
                    sf = wk.tile([128, 8], F32, tag="sf")
                    nc.scalar.copy(sf[:], s8[:])
                    ts(sf[:], sf[:], 1.0 / 64, None, A.mult)
                    vf = wk.tile([128, F], F32, tag="vf")
                    nc.scalar.copy(vf[:], i8[:])
                    nc.vector.tensor_mul(
                        out=dq[:].rearrange("p (g k) -> p g k", k=64),
                        in0=vf[:].rearrange("p (g k) -> p g k", k=64),
                        in1=sf[:, :, None].to_broadcast((128, 8, 64)))
                else:
                    sq = wk.tile([128, 1], F32, tag="sq")
                    if nrow < 128:
                        nc.any.memset(sq[:], 1.0)
                    ssrc = vt[vnr * F + 4 * rr:vnr * F + 4 * (rr + nrow)] \
                        .bitcast(F32).rearrange("(p n) -> p n", n=1)
                    nc.sync.dma_start(sq[a:b, :], ssrc)
                    nc.scalar.activation(dq[:], i8[:],
                                         mybir.ActivationFunctionType.Copy,
                                         scale=sq[:, :])
                _transpose_in(vT, dq, tt)

            # ---- DRAM scratch
            # vproj rows j = w3-projection of v_in row j; v rows outside the
            # sequence are int8 zeros (scale 1) and project to exact zeros
            vproj = dp.tile([TCV, F], F16)
            # apad: 1 guard row + TC data rows + 1 guard row, row = [8 x 128]
            apad = dp.tile([TC + 2, H * W], F16)

            # zero tile for apad guards
            z_t = pers.tile([128, H * W], F16, tag="zt")
            nc.any.memzero(z_t[:])
            nc.sync.dma_start(apad[0:1, :], z_t[0:1, :])
            nc.sync.dma_start(apad[TC + 1:TC + 2, :], z_t[0:1, :])

            # ---- persistent SBUF activations
            qrT = pers.tile([128, KF, TC], F16, tag="qrT")  # relu(q@w1) f-major
            xT = pers.tile([128, KF, TC], F16, tag="xT")    # band out, f-major

            # ================= Phase A: q-proj + relu (f-major out) ===========
            for fo in range(KF):
                for b0 in range(0, TC, 512):
                    bw = min(512, TC - b0)
                    ps = psm.tile([128, 512], F32, tag="mm")
                    for k in range(KF):
                        nc.tensor.matmul(
                            ps[:, 0:bw], w1_t[:, k, fo * 128:(fo + 1) * 128],
                            qT[:, k, b0:b0 + bw],
                            start=(k == 0), stop=(k == KF - 1))
                    nc.scalar.activation(qrT[:, fo, b0:b0 + bw],
                                         ps[:, 0:bw],
                                         mybir.ActivationFunctionType.Relu)

            # ================= Phase C: v-proj (t-major out) -> vproj =========
            for tb in range(NTV):
                ps = psm.tile([128, 512], F32, tag="mm")
                for k in range(KF):
                    nc.tensor.matmul(
                        ps[:], vT[:, k, tb * 128:(tb + 1) * 128],
                        w3_t[:, k, :],
                        start=(k == 0), stop=(k == KF - 1))
                v_sb = wk.tile([128, F], F16, tag="vsb")
                nc.scalar.copy(v_sb[:], ps[:])
                nc.sync.dma_start(vproj[tb * 128:(tb + 1) * 128, :], v_sb[:])

            # ====== Phase B: s-proj (t-major) + softmax -> apad (padded) ======
            for tb in range(NT):
                ps = psm.tile([128, 512], F32, tag="mm")
                for k in range(KF):
                    nc.tensor.matmul(
                        ps[:, 0:HC], qrT[:, k, tb * 128:(tb + 1) * 128],
                        w2_t[:, k, :],
                        start=(k == 0), stop=(k == KF - 1))
                e_t = wk.tile([128, HC], F32, tag="et")
                nc.scalar.activation(e_t[:], ps[:, 0:HC],
                                     mybir.ActivationFunctionType.Exp)
                zs = wk.tile([128, H], F32, tag="zs")
                nc.vector.reduce_sum(zs[:],
                                     e_t[:].rearrange("p (h c) -> p h c", c=C),
                                     axis=mybir.AxisListType.X)
                rz = wk.tile([128, H], F32, tag="rz")
                nc.vector.reciprocal(rz[:], zs[:])
                ap_t = wk.tile([128, H * W], F16, tag="apad")
                if tb < 2:
                    # zero the pad region once per pool slot (bufs=2); the pad
                    # columns are never overwritten afterwards
                    nc.any.memzero(ap_t[:])
                nc.vector.tensor_mul(
                    out=ap_t[:].rearrange("p (h w) -> p h w", w=W)[:, :, 0:C],
                    in0=e_t[:].rearrange("p (h c) -> p h c", c=C),
                    in1=rz[:, :, None].to_broadcast((128, H, C)))
                nc.sync.dma_start(apad[1 + tb * 128:1 + (tb + 1) * 128, :],
                                  ap_t[:])

            # ================= Phase D: banded attention matmuls ==============
            # x[t', h*64+d] = sum_s vproj[VOFF+t0+s, h*64+d] * B_h[s, t']
            # B_h loaded via transpose-DMA of sheared apad view.
            apad_h = apad.tensor  # underlying DRAM handle
            apad_off = apad.offset if isinstance(apad.offset, int) else 0
            for g in range(NB // 4):    # groups of 4 band blocks = 256 t'
                pss = [psb.tile([128, 512], F32, tag="px", name=f"px{g}_{pi}")
                       for pi in range(4)]
                for j in range(4):
                    bi = g * 4 + j
                    t0 = S * bi
                    vsp = wk.tile([128, F], F16, tag="vsp")
                    nc.sync.dma_start(vsp[:],
                                      vproj[VOFF + t0:VOFF + t0 + 128, :])
                    for p in range(4):      # head pairs
                        for i in range(2):
                            h = 2 * p + i
                            b_t = bp.tile([W, S], F16, tag="bt")
                            src = bass.AP(
                                tensor=apad_h,
                                offset=apad_off + (1 + t0) * (H * W) + h * W,
                                ap=[[H * W - 1, S], [1, W]])
                            eng = nc.scalar if h % 2 else nc.sync
                            eng.dma_start_transpose(b_t[:], src)
                            # lhsT = v head-pair [128, 128]; valid out rows are
                            # [i*64:(i+1)*64]; the other half is garbage and
                            # ignored at copyback.
                            nc.tensor.matmul(
                                pss[p][:, j * 128 + i * 64:
                                       j * 128 + (i + 1) * 64],
                                vsp[:, p * 128:(p + 1) * 128], b_t[:],
                                start=True, stop=True)
                # copy valid quadrants -> xT (f-major): fold p rows 0:63 = head
                # 2p (cols i=0), rows 64:127 = head 2p+1 (cols i=1)
                for p in range(4):
                    ps3 = pss[p][:].rearrange("d (j i k) -> d j i k", j=4, i=2)
                    dst = xT[:, p, g * 256:(g + 1) * 256] \
                        .rearrange("d (j k) -> d j k", j=4)
                    nc.vector.tensor_copy(out=dst[0:64], in_=ps3[0:64, :, 0, :])
                    nc.vector.tensor_copy(out=dst[64:128],
                                          in_=ps3[64:128, :, 1, :])

            # ========= Phase E: out-proj + per-row int8 quantization ==========
            for tb in range(NT):
                ps = psm.tile([128, 512], F32, tag="mm")
                for k in range(KF):
                    nc.tensor.matmul(
                        ps[:], xT[:, k, tb * 128:(tb + 1) * 128],
                        wo_t[:, k, :],
                        start=(k == 0), stop=(k == KF - 1))
                am = wk.tile([128, 1], F32, tag="am")
                nc.vector.reduce_max(am[:], ps[:], axis=mybir.AxisListType.X,
                                     apply_absolute_value=True)
                rz = wk.tile([128, 1], F32, tag="orz")
                nc.vector.reciprocal(rz[:], am[:])
                rs = wk.tile([128, 1], F32, tag="ors")
                nc.vector.tensor_scalar_mul(rs[:], rz[:], ODN)
                y = wk.tile([128, F], F32, tag="oy")
                nc.scalar.activation(y[:], ps[:],
                                     mybir.ActivationFunctionType.Copy,
                                     scale=rs[:, :])
                ost = wk.tile([128, 1], F32, tag="ost")
                nc.vector.tensor_scalar_mul(ost[:], am[:], 1.0 / ODN)
                nc.scalar.dma_start(os_[tb * 128:(tb + 1) * 128, :], ost[:])
                if not OUT7:
                    # round to nearest int (RNE) via magic add/sub, then
                    # convert: the value is exactly integral so the
                    # convert's rounding mode is irrelevant
                    yr = wk.tile([128, F], F32, tag="oyr")
                    ts(yr[:], y[:], MAGIC, -MAGIC, A.add, A.add)
                    oqt = wk.tile([128, F], I8, tag="oqt")
                    nc.vector.tensor_copy(out=oqt[:], in_=yr[:])
                    nc.sync.dma_start(oq[tb * 128:(tb + 1) * 128, :], oqt[:])
                    continue
                # biased 7-bit codes in [1,125]: rint via magic, +63 folded
                # into the second (still exact-integer) add
                yr = wk.tile([128, F], F32, tag="oyr")
                ts(yr[:], y[:], MAGIC, 63.0 - MAGIC, A.add, A.add)
                # pack 8 codes -> 7 bytes: B_j = c_j + 128*bit_j(c7)
                c8 = yr[:].rearrange("p (k j) -> p k j", j=8)
                pko = wk.tile([128, 7 * 64], F32, tag="pko")
                b0 = wk.tile([128, 64], F32, tag="pb0")
                b1 = wk.tile([128, 64], F32, tag="pb1")
                tbit = wk.tile([128, 64], F32, tag="tbit")
                nc.vector.tensor_copy(out=b0[:], in_=c8[:, :, 7])
                bs, bd = b0, b1
                for j in range(7):
                    # bd = floor(bs/2); fractions are {0,.5} so -0.25 rounds
                    ts(bd[:], bs[:], 0.5, -0.25, A.mult, A.add)
                    ts(bd[:], bd[:], MAGIC, -MAGIC, A.add, A.add)
                    ts(tbit[:], bd[:], -2.0, None, A.mult)
                    nc.vector.tensor_add(out=tbit[:], in0=tbit[:],
                                         in1=bs[:])
                    ts(tbit[:], tbit[:], 128.0, None, A.mult)
                    nc.vector.tensor_add(out=pko[:, j * 64:(j + 1) * 64],
                                         in0=tbit[:], in1=c8[:, :, j])
                    bs, bd = bd, bs
                oqt = wk.tile([128, 7 * 64], U8, tag="oqt7")
                nc.vector.tensor_copy(out=oqt[:], in_=pko[:])
                nc.sync.dma_start(
                    oq[tb * 128:(tb + 1) * 128, :].bitcast(U8), oqt[:])

    nc.compile()
    return nc
'''

exec(compile(_BUILD_SRC, "bass_build_k", "exec"), globals())


def _make_exec(nc, devices):
    """Cached jitted executable for one bass module; outputs come from
    donated on-device zero buffers (mkzeros)."""
    import jax
    import jax.numpy as jnp
    from jax.sharding import Mesh, PartitionSpec, NamedSharding
    from jax.experimental.shard_map import shard_map
    from concourse.bass2jax import _bass_exec_p, partition_id_tensor

    partition_name = (nc.partition_id_tensor.name
                      if nc.partition_id_tensor else None)
    in_names, out_names, out_avals = [], [], []
    for alloc in nc.m.functions[0].allocations:
        if not isinstance(alloc, mybir.MemoryLocationSet):
            continue
        if alloc.kind not in ("ExternalInput", "ExternalOutput"):
            continue
        name = alloc.memorylocations[0].name
        if alloc.kind == "ExternalInput":
            if name != partition_name:
                in_names.append(name)
        else:
            out_avals.append(jax.core.ShapedArray(
                tuple(alloc.tensor_shape), mybir.dt.np(alloc.dtype)))
            out_names.append(name)
    n_params, n_outs = len(in_names), len(out_avals)
    in_names_all = list(in_names) + list(out_names)
    if partition_name is not None:
        in_names_all.append(partition_name)

    def _body(*args):
        operands = list(args)
        if partition_name is not None:
            operands.append(partition_id_tensor())
        return tuple(_bass_exec_p.bind(
            *operands,
            out_avals=tuple(out_avals),
            in_names=tuple(in_names_all),
            out_names=tuple(out_names),
            lowering_input_output_aliases=(),
            sim_require_finite=True,
            sim_require_nnan=True,
            nc=nc))

    n = len(devices)
    mesh = Mesh(np.asarray(devices), ("core",))
    shard = NamedSharding(mesh, PartitionSpec("core"))
    n_args = n_params + n_outs
    donate = tuple(range(n_params, n_args))
    mkzeros = jax.jit(
        lambda: tuple(jnp.zeros((n * a.shape[0], *a.shape[1:]), a.dtype)
                      for a in out_avals),
        out_shardings=(shard,) * n_outs)
    in_specs = (PartitionSpec("core"),) * n_args
    out_specs = (PartitionSpec("core"),) * n_outs
    sharded = jax.jit(
        shard_map(_body, mesh=mesh, in_specs=in_specs, out_specs=out_specs,
                  check_rep=False),
        donate_argnums=donate, keep_unused=True)
    return {"sharded": sharded, "mkzeros": mkzeros, "in_names": in_names,
            "out_names": out_names, "shard": shard, "n": n}


def _get_state():
    if "state" in _CACHE:
        return _CACHE["state"]
    import jax
    from concurrent.futures import ThreadPoolExecutor
    from concourse.bass2jax import install_neuronx_cc_hook
    install_neuronx_cc_hook()
    devices = jax.devices()[:B]
    wexec = _make_exec(_build_w(), devices)
    kexecs = {}
    for c0, tc in CHUNKS:
        kexecs[(tc, c0)] = _make_exec(_build_k(tc, c0), devices)
    state = {"w": wexec, "k": kexecs,
             "pool": ThreadPoolExecutor(max_workers=3)}
    _CACHE["state"] = state
    return state


def _to_bf16_f32(x32):
    """fp32 -> bf16 via round-half-up on the upper 16 bits, returned as
    exact f32 values (so host and device share bit-identical scales)."""
    x32 = np.ascontiguousarray(x32, np.float32)
    tmp = x32.view(np.uint32) + np.uint32(0x8000)
    np.bitwise_and(tmp, np.uint32(0xFFFF0000), out=tmp)
    return tmp.view(np.float32)


def _quant_into(x, codes, scales, ybuf):
    """Per-row symmetric int8 quantization of (B, n, F) fp32 into
    preallocated codes (int8) and scales (f32, bf16-exact) views."""
    n = x.shape[1]
    a = np.maximum(x.max(axis=-1), -x.min(axis=-1))
    np.maximum(a, 1e-30, out=a)
    s = _to_bf16_f32(a / QD)
    y = ybuf[:, :n]
    np.multiply(x, (1.0 / s)[..., None], out=y)
    np.rint(y, out=y)
    codes[...] = y          # values are exactly integral: cast is exact
    scales[...] = s


def _quant6_into(x, codes, scales, ybuf):
    """Per-row 6-bit quantization of (B, n, F) fp32, packed 4 codes -> 3
    bytes in the plane layout the device kernel unpacks."""
    n = x.shape[1]
    a = np.maximum(x.max(axis=-1), -x.min(axis=-1))
    np.maximum(a, 1e-30, out=a)
    s = _to_bf16_f32(a / QD6)
    y = ybuf[:, :n]
    np.multiply(x, (1.0 / s)[..., None], out=y)
    np.rint(y, out=y)
    y += 32.0                   # biased codes in [2, 62]
    c = y.astype(np.uint8).reshape(-1, n, 128, 4)
    b3 = c[..., 3]
    cu = codes.view(np.uint8)
    cu[:, :, 0:128] = c[..., 0] + ((b3 & 3) << 6)
    cu[:, :, 128:256] = c[..., 1] + (((b3 >> 2) & 3) << 6)
    cu[:, :, 256:384] = c[..., 2] + ((b3 >> 4) << 6)
    scales[...] = s


def _quant5_into(x, codes, scales, ybuf):
    """Per-row 5-bit quantization of (B, n, F) fp32, packed 8 codes -> 5
    bytes in the plane layout the device kernel unpacks."""
    n = x.shape[1]
    a = np.maximum(x.max(axis=-1), -x.min(axis=-1))
    np.maximum(a, 1e-30, out=a)
    s = _to_bf16_f32(a / QD5)
    y = ybuf[:, :n]
    np.multiply(x, (1.0 / s)[..., None], out=y)
    np.rint(y, out=y)
    y += 16.0                   # biased codes in [1, 31]
    c = y.astype(np.uint8).reshape(-1, n, 64, 8)
    G = (c[..., 5].astype(np.uint16) + (c[..., 6].astype(np.uint16) << 5)
         + (c[..., 7].astype(np.uint16) << 10))
    cu = codes.view(np.uint8)
    for j in range(5):
        cu[:, :, j * 64:(j + 1) * 64] = \
            c[..., j] + (((G >> (3 * j)) & 7).astype(np.uint8) << 5)
    scales[...] = s


def _quant_vblk_into(x, codes, scales_u8, ybuf):
    """Block-64 int8 quantization of (B, n, F) fp32; 8 fp8 scales per row,
    shipped pre-multiplied by 64 (fp8 normal range), dequant divides."""
    import ml_dtypes
    n = x.shape[1]
    xb = x.reshape(-1, n, F // 64, 64)
    a = np.maximum(xb.max(axis=-1), -xb.min(axis=-1))
    np.maximum(a, 0.25, out=a)
    s8 = (a * (64.0 / VDN)).astype(ml_dtypes.float8_e4m3)
    s = s8.astype(np.float32) * (1.0 / 64)
    y = ybuf[:, :n].reshape(-1, n, F // 64, 64)
    np.multiply(xb, (1.0 / s)[..., None], out=y)
    np.rint(y, out=y)
    codes.reshape(-1, n, F // 64, 64)[...] = y
    scales_u8[...] = s8.view(np.uint8)


def _weights_device(st, w1, w2, w3, w_out):
    """Device-resident fp16 weights, re-uploaded only when contents change."""
    import jax
    ws = [np.ascontiguousarray(np.asarray(w), np.float32)
          for w in (w1, w2, w3, w_out)]
    cached = _CACHE.get("wfull")
    if cached is not None and all(
            np.array_equal(a, b) for a, b in zip(ws, _CACHE["whost"])):
        return cached
    wx = st["w"]
    wzeros = wx["mkzeros"]()
    arrs = {"w1s": ws[0], "w2s": ws[1], "w3s": ws[2], "wos": ws[3]}
    wouts = wx["sharded"](*[arrs[n] for n in wx["in_names"]], *wzeros)
    wfull = dict(zip(wx["out_names"], wouts))
    _CACHE["whost"] = ws
    _CACHE["wfull"] = wfull
    return wfull


def kernel(query, key, value, w1, w2, w3, w_out, _trace=False):
    out, ok = _kernel_once(query, key, value, w1, w2, w3, w_out)
    # The remote runtime very occasionally returns a stale/corrupt buffer.
    # Output row scales from a real run are all in (0, ~2e-3); a stale
    # (zero-initialized or garbage) buffer fails this. Retry once.
    if not ok:
        out, ok = _kernel_once(query, key, value, w1, w2, w3, w_out)
    return out


def _kernel_once(query, key, value, w1, w2, w3, w_out):
    import jax, os, time
    st = _get_state()
    put = jax.device_put
    pool = st["pool"]
    timing = os.environ.get("KTIMING")
    tt0 = time.perf_counter()
    lap = lambda tag: timing and print(
        f"  [{tag}] {time.perf_counter() - tt0:.3f}s", flush=True)

    query = np.asarray(query)
    value = np.asarray(value)

    wfull = _weights_device(st, w1, w2, w3, w_out)
    # scratch fp32 buffer shared by all quantizations (v uses all T rows)
    ybuf = _CACHE.get("ybuf")
    if ybuf is None:
        ybuf = _CACHE["ybuf"] = np.empty((B, T, F), np.float32)
    lap("weights")

    def _vpart(row0, nrows):
        blob = np.empty((B, nrows * F + NVS * nrows), np.int8)
        codes = blob[:, :nrows * F].reshape(B, nrows, F)
        vsl = value[:, row0:row0 + nrows]
        if VBLK:
            scales = blob[:, nrows * F:].view(np.uint8).reshape(B, nrows, 8)
            _quant_vblk_into(vsl, codes, scales, ybuf)
        else:
            scales = blob[:, nrows * F:].view(np.float32)
            _quant_into(vsl, codes, scales, ybuf)
        return pool.submit(put, blob.reshape(-1), shard)

    pending = []
    vfuts = {}
    for i, (c0, tc) in enumerate(CHUNKS):
        kx = st["k"][(tc, c0)]
        shard = kx["shard"]
        zeros_f = pool.submit(kx["mkzeros"])
        # q blob: codes then f32 row scales as raw bytes; ship it while
        # v is still being quantized
        qblob = np.empty((B, tc * QROW + 4 * tc), np.int8)
        qcodes = qblob[:, :tc * QROW].reshape(B, tc, QROW)
        qscales = qblob[:, tc * QROW:].view(np.float32)
        qfun = {5: _quant5_into, 6: _quant6_into, 8: _quant_into}[QBITS]
        qfun(query[:, c0:c0 + tc], qcodes, qscales, ybuf)
        qb_f = pool.submit(put, qblob.reshape(-1), shard)
        lap(f"qput{i}")
        # ship each v part just before the first launch that needs it
        for name in kx["in_names"]:
            if name.startswith("vp") and name not in vfuts:
                pi = int(name[2:])
                vfuts[name] = _vpart(VBOUNDS[pi], VBOUNDS[pi + 1] - VBOUNDS[pi])
                lap(f"{name}put")
        arrays = {"qb": qb_f.result(),
                  "w1f": wfull["w1f"], "w2f": wfull["w2f"],
                  "w3f": wfull["w3f"], "wof": wfull["wof"]}
        for name, fut in vfuts.items():
            arrays[name] = fut.result()
        ins = [arrays[n] for n in kx["in_names"]]
        outs = kx["sharded"](*ins, *zeros_f.result())
        for o_ in outs:
            o_.copy_to_host_async()   # start D2H as soon as exec finishes
        lap(f"launch{i}")
        pending.append(dict(zip(kx["out_names"], outs)))

    # ---- collect: unpack/dequantize rows with their f32 scales
    final = np.empty((B, T, F), np.float32)
    ok = True
    for ci, ((c0, tc), outs) in enumerate(zip(CHUNKS, pending)):
        oq = np.asarray(outs["oq"]).reshape(B, tc, OROW)
        os_ = np.asarray(outs["os"]).reshape(B, tc, 1)
        lap(f"fetch{ci}")
        fv = final[:, c0:c0 + tc]
        if OUT7:
            raw = oq.view(np.uint8).reshape(B, tc, 7, 64)
            low = raw & np.uint8(127)
            bits = raw >> np.uint8(7)
            ct = np.empty((B, tc, 64, 8), np.uint8)
            for j in range(7):
                ct[..., j] = low[:, :, j]
            c7 = bits[:, :, 0]
            for j in range(1, 7):
                c7 = c7 + (bits[:, :, j] << np.uint8(j))
            ct[..., 7] = c7
            fv[...] = ct.reshape(B, tc, F)
            fv -= 63.0
            fv *= os_
        else:
            np.multiply(oq, os_, out=fv)
        smax = os_.max()
        smin = os_.min()
        if not (np.isfinite(smax) and 0.0 < smin and smax < 0.1):
            ok = False
    lap("dequant")
    return final, ok


# revision 53
# speedup vs baseline: 1.0600x; 1.0600x over previous
"""Trainium2 Bass kernel for LocalDenseSynthesizerAttention.

Data-parallel over batch B=8 -> 8 cores, one batch each. The axon tunnel
(~45MB/s, effectively half-duplex) dominates, so the design minimizes wire
bytes (~21.8MB/call vs 43.4MB for the fp8/bf16 baseline; ~2.1x faster):
  - q shipped as packed 5-bit codes (8 codes -> 5 bytes) with per-row
    bf16-exact scales: the attention path is heavily damped (softmax over a
    45-wide window of tiny logits), so 5 bits cost only ~2e-3 of rel-err
  - v shipped int8 with per-64-block fp8 scales (pre-scaled x64 into fp8's
    normal range); v ships ONCE as a single zero-padded tensor shared by all
    chunk launches (window is local, pad=22), no per-chunk halo duplication
  - output quantized ON DEVICE to packed 7-bit codes (8 codes -> 7 bytes)
    with per-row f32 scales, unpacked/dequantized on host
  - all pack/unpack on device uses exact fp32 integer arithmetic (u8->f32
    convert, power-of-2 scaled floor via the +1.5*2^23 magic-add round), so
    device decode is bit-identical to the host's integer packing and
    independent of any convert-instruction rounding mode
  - projection weights shipped f32 ONCE (content-compared per call, reuses
    device-resident copies), AllGathered from 8-way shards on device, stored
    pre-transposed [128, KF, N] fp16 for the compute launches
  - device compute in fp16 (PE supports fp16 matmul): the extra mantissa
    bits vs bf16 pay for the int8/int5 wire budget; total measured rel-err
    1.63e-2 (gate 2e-2), bit-reproducible and matching the numpy simulation
  - codes+scales packed into one buffer per tensor (few device_puts; each
    put has a large fixed cost), puts dispatched from a small thread pool
  - compute split into 4 sequence chunks, one 8-core launch per chunk, so
    host quantization, uploads, exec, downloads and host dequant pipeline

The local window C=45 weighted sum is computed as banded matmuls: the banded
matrix B[s,t'] = attn[t0+t',h,s-t'] is an affine strided view of a zero-padded
attn tensor in DRAM, loaded matmul-ready via XBAR transpose-DMA.

Env knobs (defaults are the shipped config): KQBITS=5|6|8, KVBLK=1|0,
KOUT7=1|0, KCHUNKS=csv of chunk lengths, KTIMING=1 for per-phase timings.
NOTE: KQBITS=6 with KOUT7=1 miscompiles (a scheduling hazard corrupts the
last tile); both features are individually fine and the default q5+out7
combination is validated end-to-end.

Self-contained: hardcodes shapes from the problem spec.
"""
import sys
sys.path.insert(0, '/opt/trn_rl_repo')
import numpy as np

import concourse.bass as bass
import concourse.mybir as mybir
import concourse.tile as tile
from concourse import bacc
from concourse import masks

T, F = 2048, 512
H, C, DK = 8, 45, 64
HC = H * C          # 360
W = 128             # padded attn width per head (covers s-t' in [-63,127])
S = 64              # t' band-block size
PADV = 22           # (C-1)//2
KF = F // 128       # 4 contraction chunks
B = 8               # total batches / cores
FSH = F // B        # 64 weight-shard rows per core

VH = 64             # v halo rows each side (>= PADV, keeps tiles 128-aligned)
VOFF = VH - PADV    # chunk-vpad[r] = v_logical[r + VOFF]
CHUNKS = [(0, 512), (512, 512), (1024, 512), (1536, 512)]
import os as _os
if _os.environ.get("KCHUNKS"):
    _ls = [int(x) for x in _os.environ["KCHUNKS"].split(",")]
    assert sum(_ls) == T
    CHUNKS = []
    _c = 0
    for _l in _ls:
        CHUNKS.append((_c, _l))
        _c += _l

F16 = mybir.dt.float16
F32 = mybir.dt.float32
I8 = mybir.dt.int8
U8 = mybir.dt.uint8
FP8 = mybir.dt.float8e4
QD = 126.0          # int8 quant denominator (126 leaves headroom for the
                    # bf16 round-down of the scale: 126*1.002 < 126.5)
QD6 = 30.0          # 6-bit quant denominator (codes in [-30, 30])
QD5 = 15.0          # 5-bit quant denominator (codes in [-15, 15])
VDN = 118.0         # v block-quant denominator (fp8 scale round-down can
                    # inflate codes by up to 6.25%: 118*1.0625 < 127)
MAGIC = 12582912.0  # 1.5 * 2^23: fp32 add rounds the value to nearest int
QBITS = int(_os.environ.get("KQBITS", "5"))   # q wire precision: 5, 6, or 8
QROW = {5: 320, 6: 384, 8: 512}[QBITS]        # q bytes per row on the wire
VBLK = _os.environ.get("KVBLK", "1") == "1"   # v block-64 quant (else row)
NVS = 8 if VBLK else 4   # v scale bytes per row (8 fp8 / one f32)
OUT7 = _os.environ.get("KOUT7", "1") == "1"   # output packed 7-bit
# v ships as several tensors split at these rows (each 64 mod 128, so no
# halo-offset tile straddles a boundary); a chunk launch gates only on the
# parts it reads, so early chunks' downloads overlap later uploads
VBOUNDS = [0, 576, 1088, T]
OROW = 448 if OUT7 else 512                   # output bytes per row
ODN = 62.0 if OUT7 else QD                    # output quant denominator

_CACHE = {}


# The build functions are compiled from a synthetic filename so the
# source-location debug info embedded in the BIR (and thus the NEFF
# cache key) does not depend on where this file lives.
_BUILD_SRC = r'''
def _build_w():
    """Weights launch (first call only): AllGather 8-way f32 weight shards,
    convert to fp16 in the matmul-ready [128, KF, N] layout, store to
    device-resident DRAM outputs."""
    nc = bacc.Bacc("TRN2", target_bir_lowering=False, debug=False,
                   num_devices=B, disable_frame_to_traceback=True)
    w1s = nc.dram_tensor("w1s", (FSH, F), F32, kind="ExternalInput")
    w2s = nc.dram_tensor("w2s", (FSH, HC), F32, kind="ExternalInput")
    w3s = nc.dram_tensor("w3s", (FSH, F), F32, kind="ExternalInput")
    wos = nc.dram_tensor("wos", (FSH, F), F32, kind="ExternalInput")
    w1f = nc.dram_tensor("w1f", (128, KF * F), F16, kind="ExternalOutput")
    w2f = nc.dram_tensor("w2f", (128, KF * HC), F16, kind="ExternalOutput")
    w3f = nc.dram_tensor("w3f", (128, KF * F), F16, kind="ExternalOutput")
    wof = nc.dram_tensor("wof", (128, KF * F), F16, kind="ExternalOutput")
    groups = [list(range(B))]
    with tile.TileContext(nc) as tc:
        with tc.tile_pool(name="dram", bufs=1, space="DRAM") as dp, \
             tc.tile_pool(name="sb", bufs=2) as sp:
            for idx, (shard, out, n) in enumerate(
                    ((w1s, w1f, F), (w2s, w2f, HC),
                     (w3s, w3f, F), (wos, wof, F))):
                stage = dp.tile([FSH, n], F32, name=f"st{idx}")
                full = dp.tile([F, n], F32, name=f"fu{idx}")
                # collectives cannot read IO tensors: stage shards first
                nc.sync.dma_start(stage[:, :], shard[:, :])
                nc.gpsimd.collective_compute(
                    "AllGather", mybir.AluOpType.bypass, groups,
                    [stage[:, :]], [full[:, :]])
                sb32 = sp.tile([128, KF, n], F32, tag=f"sb32_{n}")
                nc.sync.dma_start(
                    sb32[:], full[:, :].rearrange("(ko p) n -> p ko n", p=128))
                sb16 = sp.tile([128, KF, n], F16, tag=f"sb16_{n}")
                nc.scalar.copy(sb16[:], sb32[:])
                nc.sync.dma_start(
                    out[:, :].rearrange("p (ko n) -> p ko n", ko=KF), sb16[:])
    nc.compile()
    return nc


def _build_k(TC, C0):
    """Compute launch for one sequence chunk of TC rows starting at C0."""
    TCV = TC + 2 * VH           # v rows incl halo
    NT = TC // 128              # t-tiles in the chunk
    NTV = TCV // 128            # v tiles incl halo
    NB = TC // S                # band blocks
    nc = bacc.Bacc("TRN2", target_bir_lowering=False, debug=False,
                   num_devices=B, disable_frame_to_traceback=True)
    # qb = q rows (TC x QROW bytes: int8 codes, or 5/6-bit codes packed in
    # byte planes) then TC f32 row scales (raw bytes);
    # v parts (shared by all chunk launches), each codes-then-scales; a
    # chunk declares only the parts it reads, so its launch is not gated on
    # the other parts' uploads. Logical padded row r maps to v row r - VH;
    # the VH-row sequence-edge pads are zero-filled on device, not shipped.
    qb = nc.dram_tensor("qb", (TC * QROW + 4 * TC,), I8, kind="ExternalInput")
    vparts = []            # (tensor, row0, nrows) for the declared parts
    r_lo = max(C0 - VH, 0)                    # first v row this chunk reads
    r_hi = min(C0 + TC + VH, T)               # one past the last
    for pi in range(len(VBOUNDS) - 1):
        b0, b1 = VBOUNDS[pi], VBOUNDS[pi + 1]
        if r_lo < b1 and r_hi > b0:
            nn = b1 - b0
            vt = nc.dram_tensor(f"vp{pi}", (nn * F + NVS * nn,), I8,
                                kind="ExternalInput")
            vparts.append((vt, b0, nn))
    w1f = nc.dram_tensor("w1f", (128, KF * F), F16, kind="ExternalInput")
    w2f = nc.dram_tensor("w2f", (128, KF * HC), F16, kind="ExternalInput")
    w3f = nc.dram_tensor("w3f", (128, KF * F), F16, kind="ExternalInput")
    wof = nc.dram_tensor("wof", (128, KF * F), F16, kind="ExternalInput")
    oq = nc.dram_tensor("oq", (TC, OROW), I8, kind="ExternalOutput")
    os_ = nc.dram_tensor("os", (TC, 1), F32, kind="ExternalOutput")

    with tile.TileContext(nc) as tc:
        with tc.tile_pool(name="wpool", bufs=1) as wp, \
             tc.tile_pool(name="inpool", bufs=1) as inp, \
             tc.tile_pool(name="persist", bufs=1) as pers, \
             tc.tile_pool(name="work", bufs=2) as wk, \
             tc.tile_pool(name="band", bufs=4) as bp, \
             tc.tile_pool(name="psmain", bufs=2, space="PSUM") as psm, \
             tc.tile_pool(name="psband", bufs=4, space="PSUM") as psb, \
             tc.tile_pool(name="pstp", bufs=2, space="PSUM") as ptp, \
             tc.tile_pool(name="drampool", bufs=1, space="DRAM") as dp:

            # ---- weights to SBUF, [128, KF, n] fp16 (partition = contraction)
            w1_t = wp.tile([128, KF, F], F16, tag="w1")
            nc.sync.dma_start(
                w1_t[:], w1f[:, :].rearrange("p (ko n) -> p ko n", ko=KF))
            w2_t = wp.tile([128, KF, HC], F16, tag="w2")
            nc.sync.dma_start(
                w2_t[:], w2f[:, :].rearrange("p (ko n) -> p ko n", ko=KF))
            w3_t = wp.tile([128, KF, F], F16, tag="w3")
            nc.sync.dma_start(
                w3_t[:], w3f[:, :].rearrange("p (ko n) -> p ko n", ko=KF))
            wo_t = wp.tile([128, KF, F], F16, tag="wo")
            nc.sync.dma_start(
                wo_t[:], wof[:, :].rearrange("p (ko n) -> p ko n", ko=KF))

            ident = pers.tile([128, 128], F16, tag="ident")
            masks.make_identity(nc, ident[:])

            # ---- dequantize q and v (t-major int8 -> fp16), PE-transpose to
            # f-major [128 f, KF, t]
            qT = inp.tile([128, KF, TC], F16, tag="qT")
            vT = inp.tile([128, KF, TCV], F16, tag="vT")

            def _scale_ap(src_t, nrow, nbytes, tt):
                return src_t[nrow * nbytes + 4 * tt * 128:
                             nrow * nbytes + 4 * (tt + 1) * 128] \
                    .bitcast(F32).rearrange("(p n) -> p n", n=1)

            def _transpose_in(dst, dq, tt):
                for fo in range(KF):
                    pst = ptp.tile([128, 128], F16, tag="tp")
                    nc.tensor.transpose(
                        pst[:], dq[:, fo * 128:(fo + 1) * 128], ident[:])
                    nc.scalar.copy(dst[:, fo, tt * 128:(tt + 1) * 128],
                                   pst[:])

            A = mybir.AluOpType
            ts = nc.vector.tensor_scalar

            def _floor(out, in_, inv):
                # floor(in_/d) for small nonneg d-multiples-of-1/d: exact
                # round(in_*inv - 63/128) via the fp32 magic add
                ts(out, in_, inv, -0.4921875, A.mult, A.add)
                ts(out, out, MAGIC, -MAGIC, A.add, A.add)

            # ---- q tiles: unpack (if packed), dequant, transpose
            for tt in range(NT):
                sq = wk.tile([128, 1], F32, tag="sq")
                nc.sync.dma_start(sq[:], _scale_ap(qb, TC, QROW, tt))
                dq = wk.tile([128, F], F16, tag="dq")
                if QBITS == 8:
                    i8 = wk.tile([128, F], I8, tag="i8")
                    src = qb[tt * 128 * F:(tt + 1) * 128 * F]
                    nc.sync.dma_start(
                        i8[:], src.rearrange("(p n) -> p n", n=F))
                    nc.scalar.activation(dq[:], i8[:],
                                         mybir.ActivationFunctionType.Copy,
                                         scale=sq[:, :])
                    _transpose_in(qT, dq, tt)
                    continue
                # packed path: bytes hold a low field (6 or 5 bits) plus
                # high bits that assemble the remaining codes. All
                # arithmetic is exact fp32 on small integers.
                pk = wk.tile([128, QROW], U8, tag="pk")
                src = qb[tt * 128 * QROW:(tt + 1) * 128 * QROW].bitcast(U8)
                nc.sync.dma_start(
                    pk[:], src.rearrange("(p n) -> p n", n=QROW))
                pf = wk.tile([128, QROW], F32, tag="pf")
                nc.scalar.copy(pf[:], pk[:])
                hi = wk.tile([128, QROW], F32, tag="hi")
                lo = wk.tile([128, QROW], F32, tag="lo")
                ct = wk.tile([128, F], F32, tag="ct")
                if QBITS == 6:
                    # planes P_j (128 cols): low 6 = code of f=4k+j; high 2
                    # = bits [2j,2j+2) of the f=4k+3 code
                    _floor(hi[:], pf[:], 1.0 / 64)
                    ts(lo[:], hi[:], -64.0, None, A.mult)
                    nc.vector.tensor_add(out=lo[:], in0=lo[:], in1=pf[:])
                    c4 = ct[:].rearrange("p (k j) -> p k j", j=4)
                    for j in range(3):
                        nc.vector.tensor_copy(
                            out=c4[:, :, j], in_=lo[:, j * 128:(j + 1) * 128])
                    t1 = wk.tile([128, 128], F32, tag="t1")
                    ts(t1[:], hi[:, 128:256], 4.0, None, A.mult)
                    nc.vector.tensor_add(out=t1[:], in0=t1[:],
                                         in1=hi[:, 0:128])
                    t2 = wk.tile([128, 128], F32, tag="t2")
                    ts(t2[:], hi[:, 256:384], 16.0, None, A.mult)
                    nc.vector.tensor_add(out=c4[:, :, 3], in0=t1[:],
                                         in1=t2[:])
                    qbias = -32.0
                else:
                    # QBITS == 5: planes P_j (j<5, 64 cols): low 5 = code of
                    # f=8k+j; high 3 = bits [3j,3j+3) of G = c5|c6<<5|c7<<10
                    _floor(hi[:], pf[:], 1.0 / 32)
                    ts(lo[:], hi[:], -32.0, None, A.mult)
                    nc.vector.tensor_add(out=lo[:], in0=lo[:], in1=pf[:])
                    c8 = ct[:].rearrange("p (k j) -> p k j", j=8)
                    for j in range(5):
                        nc.vector.tensor_copy(
                            out=c8[:, :, j], in_=lo[:, j * 64:(j + 1) * 64])
                    g = wk.tile([128, 64], F32, tag="g")
                    t1 = wk.tile([128, 64], F32, tag="t1")
                    ts(g[:], hi[:, 64:128], 8.0, None, A.mult)
                    nc.vector.tensor_add(out=g[:], in0=g[:], in1=hi[:, 0:64])
                    for j, w in ((2, 64.0), (3, 512.0), (4, 4096.0)):
                        ts(t1[:], hi[:, j * 64:(j + 1) * 64], w, None, A.mult)
                        nc.vector.tensor_add(out=g[:], in0=g[:], in1=t1[:])
                    f1 = wk.tile([128, 64], F32, tag="f1")
                    _floor(f1[:], g[:], 1.0 / 32)
                    ts(t1[:], f1[:], -32.0, None, A.mult)
                    nc.vector.tensor_add(out=c8[:, :, 5], in0=t1[:],
                                         in1=g[:])
                    f2 = wk.tile([128, 64], F32, tag="f2")
                    _floor(f2[:], f1[:], 1.0 / 32)
                    ts(t1[:], f2[:], -32.0, None, A.mult)
                    nc.vector.tensor_add(out=c8[:, :, 6], in0=t1[:],
                                         in1=f1[:])
                    nc.vector.tensor_copy(out=c8[:, :, 7], in_=f2[:])
                    qbias = -16.0
                # dequant: (c + qbias) * s = c*s + qbias*s
                nbias = wk.tile([128, 1], F32, tag="nbias")
                ts(nbias[:], sq[:], qbias, None, A.mult)
                nc.scalar.activation(dq[:], ct[:],
                                     mybir.ActivationFunctionType.Identity,
                                     bias=nbias[:, :], scale=sq[:, :])
                _transpose_in(qT, dq, tt)

            # ---- v tiles: int8 dequant, transpose. Tile tt covers padded
            # rows [C0+128tt, C0+128(tt+1)) = vb rows shifted by -VH; tiles
            # overlapping the sequence edge are zero-filled then partially
            # loaded.
            for tt in range(NTV):
                p0 = C0 + tt * 128          # first padded row of the tile
                a = max(p0, VH) - p0        # first valid partition
                b = min(p0 + 128, VH + T) - p0   # one past last valid
                r0 = p0 - VH + a            # first v row
                nrow = b - a
                vt, vr0, vnr = next((t, o, nn) for t, o, nn in vparts
                                    if o <= r0 < o + nn)
                assert r0 + nrow <= vr0 + vnr   # aligned split: no straddle
                rr = r0 - vr0
                i8 = wk.tile([128, F], I8, tag="i8")
                if nrow < 128:
                    nc.any.memzero(i8[:])
                src = vt[rr * F:(rr + nrow) * F]
                nc.sync.dma_start(
                    i8[a:b, :], src.rearrange("(p n) -> p n", n=F))
                dq = wk.tile([128, F], F16, tag="dq")
                if VBLK:
                    # 8 fp8 block scales per row, pre-multiplied by 64 on
                    # the host so they sit in fp8's normal range
                    s8 = wk.tile([128, 8], FP8, tag="s8")
                    if nrow < 128:
                        # fp8 1.0 in the pad rows (codes there are 0; any
                        # finite scale works, garbage could decode to NaN)
                        nc.any.memset(s8[:], 1.0)
                    ssrc = vt[vnr * F + 8 * rr:vnr * F + 8 * (rr + nrow)] \
                        .bitcast(FP8).rearrange("(p n) -> p n", n=8)
                    nc.sync.dma_start(s8[a:b, :], ssrc)
                    sf = wk.tile([128, 8], F32, tag="sf")
                    nc.scalar.copy(sf[:], s8[:])
                    ts(sf[:], sf[:], 1.0 / 64, None, A.mult)
                    vf = wk.tile([128, F], F32, tag="vf")
                    nc.scalar.copy(vf[:], i8[:])
                    nc.vector.tensor_mul(
                        out=dq[:].rearrange("p (g k) -> p g k", k=64),
                        in0=vf[:].rearrange("p (g k) -> p g k", k=64),
                        in1=sf[:, :, None].to_broadcast((128, 8, 64)))
                else:
                    sq = wk.tile([128, 1], F32, tag="sq")
                    if nrow < 128:
                        nc.any.memset(sq[:], 1.0)
                    ssrc = vt[vnr * F + 4 * rr:vnr * F + 4 * (rr + nrow)] \
                        .bitcast(F32).rearrange("(p n) -> p n", n=1)
                    nc.sync.dma_start(sq[a:b, :], ssrc)
                    nc.scalar.activation(dq[:], i8[:],
                                         mybir.ActivationFunctionType.Copy,
                                         scale=sq[:, :])
                _transpose_in(vT, dq, tt)

            # ---- DRAM scratch
            # vproj rows j = w3-projection of v_in row j; v rows outside the
            # sequence are int8 zeros (scale 1) and project to exact zeros
            vproj = dp.tile([TCV, F], F16)
            # apad: 1 guard row + TC data rows + 1 guard row, row = [8 x 128]
            apad = dp.tile([TC + 2, H * W], F16)

            # zero tile for apad guards
            z_t = pers.tile([128, H * W], F16, tag="zt")
            nc.any.memzero(z_t[:])
            nc.sync.dma_start(apad[0:1, :], z_t[0:1, :])
            nc.sync.dma_start(apad[TC + 1:TC + 2, :], z_t[0:1, :])

            # ---- persistent SBUF activations
            qrT = pers.tile([128, KF, TC], F16, tag="qrT")  # relu(q@w1) f-major
            xT = pers.tile([128, KF, TC], F16, tag="xT")    # band out, f-major

            # ================= Phase A: q-proj + relu (f-major out) ===========
            for fo in range(KF):
                for b0 in range(0, TC, 512):
                    bw = min(512, TC - b0)
                    ps = psm.tile([128, 512], F32, tag="mm")
                    for k in range(KF):
                        nc.tensor.matmul(
                            ps[:, 0:bw], w1_t[:, k, fo * 128:(fo + 1) * 128],
                            qT[:, k, b0:b0 + bw],
                            start=(k == 0), stop=(k == KF - 1))
                    nc.scalar.activation(qrT[:, fo, b0:b0 + bw],
                                         ps[:, 0:bw],
                                         mybir.ActivationFunctionType.Relu)

            # ================= Phase C: v-proj (t-major out) -> vproj =========
            for tb in range(NTV):
                ps = psm.tile([128, 512], F32, tag="mm")
                for k in range(KF):
                    nc.tensor.matmul(
                        ps[:], vT[:, k, tb * 128:(tb + 1) * 128],
                        w3_t[:, k, :],
                        start=(k == 0), stop=(k == KF - 1))
                v_sb = wk.tile([128, F], F16, tag="vsb")
                nc.scalar.copy(v_sb[:], ps[:])
                nc.sync.dma_start(vproj[tb * 128:(tb + 1) * 128, :], v_sb[:])

            # ====== Phase B: s-proj (t-major) + softmax -> apad (padded) ======
            for tb in range(NT):
                ps = psm.tile([128, 512], F32, tag="mm")
                for k in range(KF):
                    nc.tensor.matmul(
                        ps[:, 0:HC], qrT[:, k, tb * 128:(tb + 1) * 128],
                        w2_t[:, k, :],
                        start=(k == 0), stop=(k == KF - 1))
                e_t = wk.tile([128, HC], F32, tag="et")
                nc.scalar.activation(e_t[:], ps[:, 0:HC],
                                     mybir.ActivationFunctionType.Exp)
                zs = wk.tile([128, H], F32, tag="zs")
                nc.vector.reduce_sum(zs[:],
                                     e_t[:].rearrange("p (h c) -> p h c", c=C),
                                     axis=mybir.AxisListType.X)
                rz = wk.tile([128, H], F32, tag="rz")
                nc.vector.reciprocal(rz[:], zs[:])
                ap_t = wk.tile([128, H * W], F16, tag="apad")
                if tb < 2:
                    # zero the pad region once per pool slot (bufs=2); the pad
                    # columns are never overwritten afterwards
                    nc.any.memzero(ap_t[:])
                nc.vector.tensor_mul(
                    out=ap_t[:].rearrange("p (h w) -> p h w", w=W)[:, :, 0:C],
                    in0=e_t[:].rearrange("p (h c) -> p h c", c=C),
                    in1=rz[:, :, None].to_broadcast((128, H, C)))
                nc.sync.dma_start(apad[1 + tb * 128:1 + (tb + 1) * 128, :],
                                  ap_t[:])

            # ================= Phase D: banded attention matmuls ==============
            # x[t', h*64+d] = sum_s vproj[VOFF+t0+s, h*64+d] * B_h[s, t']
            # B_h loaded via transpose-DMA of sheared apad view.
            apad_h = apad.tensor  # underlying DRAM handle
            apad_off = apad.offset if isinstance(apad.offset, int) else 0
            for g in range(NB // 4):    # groups of 4 band blocks = 256 t'
                pss = [psb.tile([128, 512], F32, tag="px", name=f"px{g}_{pi}")
                       for pi in range(4)]
                for j in range(4):
                    bi = g * 4 + j
                    t0 = S * bi
                    vsp = wk.tile([128, F], F16, tag="vsp")
                    nc.sync.dma_start(vsp[:],
                                      vproj[VOFF + t0:VOFF + t0 + 128, :])
                    for p in range(4):      # head pairs
                        for i in range(2):
                            h = 2 * p + i
                            b_t = bp.tile([W, S], F16, tag="bt")
                            src = bass.AP(
                                tensor=apad_h,
                                offset=apad_off + (1 + t0) * (H * W) + h * W,
                                ap=[[H * W - 1, S], [1, W]])
                            eng = nc.scalar if h % 2 else nc.sync
                            eng.dma_start_transpose(b_t[:], src)
                            # lhsT = v head-pair [128, 128]; valid out rows are
                            # [i*64:(i+1)*64]; the other half is garbage and
                            # ignored at copyback.
                            nc.tensor.matmul(
                                pss[p][:, j * 128 + i * 64:
                                       j * 128 + (i + 1) * 64],
                                vsp[:, p * 128:(p + 1) * 128], b_t[:],
                                start=True, stop=True)
                # copy valid quadrants -> xT (f-major): fold p rows 0:63 = head
                # 2p (cols i=0), rows 64:127 = head 2p+1 (cols i=1)
                for p in range(4):
                    ps3 = pss[p][:].rearrange("d (j i k) -> d j i k", j=4, i=2)
                    dst = xT[:, p, g * 256:(g + 1) * 256] \
                        .rearrange("d (j k) -> d j k", j=4)
                    nc.vector.tensor_copy(out=dst[0:64], in_=ps3[0:64, :, 0, :])
                    nc.vector.tensor_copy(out=dst[64:128],
                                          in_=ps3[64:128, :, 1, :])

            # ========= Phase E: out-proj + per-row int8 quantization ==========
            for tb in range(NT):
                ps = psm.tile([128, 512], F32, tag="mm")
                for k in range(KF):
                    nc.tensor.matmul(
                        ps[:], xT[:, k, tb * 128:(tb + 1) * 128],
                        wo_t[:, k, :],
                        start=(k == 0), stop=(k == KF - 1))
                am = wk.tile([128, 1], F32, tag="am")
                nc.vector.reduce_max(am[:], ps[:], axis=mybir.AxisListType.X,
                                     apply_absolute_value=True)
                rz = wk.tile([128, 1], F32, tag="orz")
                nc.vector.reciprocal(rz[:], am[:])
                rs = wk.tile([128, 1], F32, tag="ors")
                nc.vector.tensor_scalar_mul(rs[:], rz[:], ODN)
                y = wk.tile([128, F], F32, tag="oy")
                nc.scalar.activation(y[:], ps[:],
                                     mybir.ActivationFunctionType.Copy,
                                     scale=rs[:, :])
                ost = wk.tile([128, 1], F32, tag="ost")
                nc.vector.tensor_scalar_mul(ost[:], am[:], 1.0 / ODN)
                nc.scalar.dma_start(os_[tb * 128:(tb + 1) * 128, :], ost[:])
                if not OUT7:
                    # round to nearest int (RNE) via magic add/sub, then
                    # convert: the value is exactly integral so the
                    # convert's rounding mode is irrelevant
                    yr = wk.tile([128, F], F32, tag="oyr")
                    ts(yr[:], y[:], MAGIC, -MAGIC, A.add, A.add)
                    oqt = wk.tile([128, F], I8, tag="oqt")
                    nc.vector.tensor_copy(out=oqt[:], in_=yr[:])
                    nc.sync.dma_start(oq[tb * 128:(tb + 1) * 128, :], oqt[:])
                    continue
                # biased 7-bit codes in [1,125]: rint via magic, +63 folded
                # into the second (still exact-integer) add
                yr = wk.tile([128, F], F32, tag="oyr")
                ts(yr[:], y[:], MAGIC, 63.0 - MAGIC, A.add, A.add)
                # pack 8 codes -> 7 bytes: B_j = c_j + 128*bit_j(c7)
                c8 = yr[:].rearrange("p (k j) -> p k j", j=8)
                pko = wk.tile([128, 7 * 64], F32, tag="pko")
                b0 = wk.tile([128, 64], F32, tag="pb0")
                b1 = wk.tile([128, 64], F32, tag="pb1")
                tbit = wk.tile([128, 64], F32, tag="tbit")
                nc.vector.tensor_copy(out=b0[:], in_=c8[:, :, 7])
                bs, bd = b0, b1
                for j in range(7):
                    # bd = floor(bs/2); fractions are {0,.5} so -0.25 rounds
                    ts(bd[:], bs[:], 0.5, -0.25, A.mult, A.add)
                    ts(bd[:], bd[:], MAGIC, -MAGIC, A.add, A.add)
                    ts(tbit[:], bd[:], -2.0, None, A.mult)
                    nc.vector.tensor_add(out=tbit[:], in0=tbit[:],
                                         in1=bs[:])
                    ts(tbit[:], tbit[:], 128.0, None, A.mult)
                    nc.vector.tensor_add(out=pko[:, j * 64:(j + 1) * 64],
                                         in0=tbit[:], in1=c8[:, :, j])
                    bs, bd = bd, bs
                oqt = wk.tile([128, 7 * 64], U8, tag="oqt7")
                nc.vector.tensor_copy(out=oqt[:], in_=pko[:])
                nc.sync.dma_start(
                    oq[tb * 128:(tb + 1) * 128, :].bitcast(U8), oqt[:])

    nc.compile()
    return nc
'''

exec(compile(_BUILD_SRC, "bass_build_k", "exec"), globals())


def _make_exec(nc, devices):
    """Cached jitted executable for one bass module; outputs come from
    donated on-device zero buffers (mkzeros)."""
    import jax
    import jax.numpy as jnp
    from jax.sharding import Mesh, PartitionSpec, NamedSharding
    from jax.experimental.shard_map import shard_map
    from concourse.bass2jax import _bass_exec_p, partition_id_tensor

    partition_name = (nc.partition_id_tensor.name
                      if nc.partition_id_tensor else None)
    in_names, out_names, out_avals = [], [], []
    for alloc in nc.m.functions[0].allocations:
        if not isinstance(alloc, mybir.MemoryLocationSet):
            continue
        if alloc.kind not in ("ExternalInput", "ExternalOutput"):
            continue
        name = alloc.memorylocations[0].name
        if alloc.kind == "ExternalInput":
            if name != partition_name:
                in_names.append(name)
        else:
            out_avals.append(jax.core.ShapedArray(
                tuple(alloc.tensor_shape), mybir.dt.np(alloc.dtype)))
            out_names.append(name)
    n_params, n_outs = len(in_names), len(out_avals)
    in_names_all = list(in_names) + list(out_names)
    if partition_name is not None:
        in_names_all.append(partition_name)

    def _body(*args):
        operands = list(args)
        if partition_name is not None:
            operands.append(partition_id_tensor())
        return tuple(_bass_exec_p.bind(
            *operands,
            out_avals=tuple(out_avals),
            in_names=tuple(in_names_all),
            out_names=tuple(out_names),
            lowering_input_output_aliases=(),
            sim_require_finite=True,
            sim_require_nnan=True,
            nc=nc))

    n = len(devices)
    mesh = Mesh(np.asarray(devices), ("core",))
    shard = NamedSharding(mesh, PartitionSpec("core"))
    n_args = n_params + n_outs
    donate = tuple(range(n_params, n_args))
    mkzeros = jax.jit(
        lambda: tuple(jnp.zeros((n * a.shape[0], *a.shape[1:]), a.dtype)
                      for a in out_avals),
        out_shardings=(shard,) * n_outs)
    in_specs = (PartitionSpec("core"),) * n_args
    out_specs = (PartitionSpec("core"),) * n_outs
    sharded = jax.jit(
        shard_map(_body, mesh=mesh, in_specs=in_specs, out_specs=out_specs,
                  check_rep=False),
        donate_argnums=donate, keep_unused=True)
    return {"sharded": sharded, "mkzeros": mkzeros, "in_names": in_names,
            "out_names": out_names, "shard": shard, "n": n}


def _get_state():
    if "state" in _CACHE:
        return _CACHE["state"]
    import jax
    from concurrent.futures import ThreadPoolExecutor
    from concourse.bass2jax import install_neuronx_cc_hook
    install_neuronx_cc_hook()
    devices = jax.devices()[:B]
    wexec = _make_exec(_build_w(), devices)
    kexecs = {}
    for c0, tc in CHUNKS:
        kexecs[(tc, c0)] = _make_exec(_build_k(tc, c0), devices)
    state = {"w": wexec, "k": kexecs,
             "pool": ThreadPoolExecutor(max_workers=3)}
    _CACHE["state"] = state
    return state


def _to_bf16_f32(x32):
    """fp32 -> bf16 via round-half-up on the upper 16 bits, returned as
    exact f32 values (so host and device share bit-identical scales)."""
    x32 = np.ascontiguousarray(x32, np.float32)
    tmp = x32.view(np.uint32) + np.uint32(0x8000)
    np.bitwise_and(tmp, np.uint32(0xFFFF0000), out=tmp)
    return tmp.view(np.float32)


def _quant_into(x, codes, scales, ybuf):
    """Per-row symmetric int8 quantization of (B, n, F) fp32 into
    preallocated codes (int8) and scales (f32, bf16-exact) views."""
    n = x.shape[1]
    a = np.maximum(x.max(axis=-1), -x.min(axis=-1))
    np.maximum(a, 1e-30, out=a)
    s = _to_bf16_f32(a / QD)
    y = ybuf[:, :n]
    np.multiply(x, (1.0 / s)[..., None], out=y)
    np.rint(y, out=y)
    codes[...] = y          # values are exactly integral: cast is exact
    scales[...] = s


def _quant6_into(x, codes, scales, ybuf):
    """Per-row 6-bit quantization of (B, n, F) fp32, packed 4 codes -> 3
    bytes in the plane layout the device kernel unpacks."""
    n = x.shape[1]
    a = np.maximum(x.max(axis=-1), -x.min(axis=-1))
    np.maximum(a, 1e-30, out=a)
    s = _to_bf16_f32(a / QD6)
    y = ybuf[:, :n]
    np.multiply(x, (1.0 / s)[..., None], out=y)
    np.rint(y, out=y)
    y += 32.0                   # biased codes in [2, 62]
    c = y.astype(np.uint8).reshape(-1, n, 128, 4)
    b3 = c[..., 3]
    cu = codes.view(np.uint8)
    cu[:, :, 0:128] = c[..., 0] + ((b3 & 3) << 6)
    cu[:, :, 128:256] = c[..., 1] + (((b3 >> 2) & 3) << 6)
    cu[:, :, 256:384] = c[..., 2] + ((b3 >> 4) << 6)
    scales[...] = s


def _quant5_into(x, codes, scales, ybuf):
    """Per-row 5-bit quantization of (B, n, F) fp32, packed 8 codes -> 5
    bytes in the plane layout the device kernel unpacks."""
    n = x.shape[1]
    a = np.maximum(x.max(axis=-1), -x.min(axis=-1))
    np.maximum(a, 1e-30, out=a)
    s = _to_bf16_f32(a / QD5)
    y = ybuf[:, :n]
    np.multiply(x, (1.0 / s)[..., None], out=y)
    np.rint(y, out=y)
    y += 16.0                   # biased codes in [1, 31]
    c = y.astype(np.uint8).reshape(-1, n, 64, 8)
    G = (c[..., 5].astype(np.uint16) + (c[..., 6].astype(np.uint16) << 5)
         + (c[..., 7].astype(np.uint16) << 10))
    cu = codes.view(np.uint8)
    for j in range(5):
        cu[:, :, j * 64:(j + 1) * 64] = \
            c[..., j] + (((G >> (3 * j)) & 7).astype(np.uint8) << 5)
    scales[...] = s


def _quant_vblk_into(x, codes, scales_u8, ybuf):
    """Block-64 int8 quantization of (B, n, F) fp32; 8 fp8 scales per row,
    shipped pre-multiplied by 64 (fp8 normal range), dequant divides."""
    import ml_dtypes
    n = x.shape[1]
    xb = x.reshape(-1, n, F // 64, 64)
    a = np.maximum(xb.max(axis=-1), -xb.min(axis=-1))
    np.maximum(a, 0.25, out=a)
    s8 = (a * (64.0 / VDN)).astype(ml_dtypes.float8_e4m3)
    s = s8.astype(np.float32) * (1.0 / 64)
    y = ybuf[:, :n].reshape(-1, n, F // 64, 64)
    np.multiply(xb, (1.0 / s)[..., None], out=y)
    np.rint(y, out=y)
    codes.reshape(-1, n, F // 64, 64)[...] = y
    scales_u8[...] = s8.view(np.uint8)


def _weights_device(st, w1, w2, w3, w_out):
    """Device-resident fp16 weights, re-uploaded only when contents change."""
    import jax
    ws = [np.ascontiguousarray(np.asarray(w), np.float32)
          for w in (w1, w2, w3, w_out)]
    cached = _CACHE.get("wfull")
    if cached is not None and all(
            np.array_equal(a, b) for a, b in zip(ws, _CACHE["whost"])):
        return cached
    wx = st["w"]
    wzeros = wx["mkzeros"]()
    arrs = {"w1s": ws[0], "w2s": ws[1], "w3s": ws[2], "wos": ws[3]}
    wouts = wx["sharded"](*[arrs[n] for n in wx["in_names"]], *wzeros)
    wfull = dict(zip(wx["out_names"], wouts))
    _CACHE["whost"] = ws
    _CACHE["wfull"] = wfull
    return wfull


def kernel(query, key, value, w1, w2, w3, w_out, _trace=False):
    out, ok = _kernel_once(query, key, value, w1, w2, w3, w_out)
    # The remote runtime very occasionally returns a stale/corrupt buffer.
    # Output row scales from a real run are all in (0, ~2e-3); a stale
    # (zero-initialized or garbage) buffer fails this. Retry once.
    if not ok:
        out, ok = _kernel_once(query, key, value, w1, w2, w3, w_out)
    return out


def _kernel_once(query, key, value, w1, w2, w3, w_out):
    import jax, os, time
    st = _get_state()
    put = jax.device_put
    pool = st["pool"]
    timing = os.environ.get("KTIMING")
    tt0 = time.perf_counter()
    lap = lambda tag: timing and print(
        f"  [{tag}] {time.perf_counter() - tt0:.3f}s", flush=True)

    query = np.asarray(query)
    value = np.asarray(value)

    wfull = _weights_device(st, w1, w2, w3, w_out)
    # scratch fp32 buffer shared by all quantizations (v uses all T rows)
    ybuf = _CACHE.get("ybuf")
    if ybuf is None:
        ybuf = _CACHE["ybuf"] = np.empty((B, T, F), np.float32)
    lap("weights")

    def _vpart(row0, nrows):
        blob = np.empty((B, nrows * F + NVS * nrows), np.int8)
        codes = blob[:, :nrows * F].reshape(B, nrows, F)
        vsl = value[:, row0:row0 + nrows]
        if VBLK:
            scales = blob[:, nrows * F:].view(np.uint8).reshape(B, nrows, 8)
            _quant_vblk_into(vsl, codes, scales, ybuf)
        else:
            scales = blob[:, nrows * F:].view(np.float32)
            _quant_into(vsl, codes, scales, ybuf)
        return pool.submit(put, blob.reshape(-1), shard)

    pending = []
    vfuts = {}
    for i, (c0, tc) in enumerate(CHUNKS):
        kx = st["k"][(tc, c0)]
        shard = kx["shard"]
        zeros_f = pool.submit(kx["mkzeros"])
        # q blob: codes then f32 row scales as raw bytes; ship it while
        # v is still being quantized
        qblob = np.empty((B, tc * QROW + 4 * tc), np.int8)
        qcodes = qblob[:, :tc * QROW].reshape(B, tc, QROW)
        qscales = qblob[:, tc * QROW:].view(np.float32)
        qfun = {5: _quant5_into, 6: _quant6_into, 8: _quant_into}[QBITS]
        qfun(query[:, c0:c0 + tc], qcodes, qscales, ybuf)
        qb_f = pool.submit(put, qblob.reshape(-1), shard)
        lap(f"qput{i}")
        # ship each v part just before the first launch that needs it
        for name in kx["in_names"]:
            if name.startswith("vp") and name not in vfuts:
                pi = int(name[2:])
                vfuts[name] = _vpart(VBOUNDS[pi], VBOUNDS[pi + 1] - VBOUNDS[pi])
                lap(f"{name}put")
        arrays = {"qb": qb_f.result(),
                  "w1f": wfull["w1f"], "w2f": wfull["w2f"],
                  "w3f": wfull["w3f"], "wof": wfull["wof"]}
        for name, fut in vfuts.items():
            arrays[name] = fut.result()
        ins = [arrays[n] for n in kx["in_names"]]
        outs = kx["sharded"](*ins, *zeros_f.result())
        for o_ in outs:
            o_.copy_to_host_async()   # start D2H as soon as exec finishes
        lap(f"launch{i}")
        pending.append(dict(zip(kx["out_names"], outs)))

    # ---- collect: unpack/dequantize rows with their f32 scales
    final = np.empty((B, T, F), np.float32)
    ok = True
    for ci, ((c0, tc), outs) in enumerate(zip(CHUNKS, pending)):
        oq = np.asarray(outs["oq"]).reshape(B, tc, OROW)
        os_ = np.asarray(outs["os"]).reshape(B, tc, 1)
        lap(f"fetch{ci}")
        fv = final[:, c0:c0 + tc]
        if OUT7:
            raw = oq.view(np.uint8).reshape(B, tc, 7, 64)
            low = raw & np.uint8(127)
            bits = raw >> np.uint8(7)
            ct = np.empty((B, tc, 64, 8), np.uint8)
            for j in range(7):
                ct[..., j] = low[:, :, j]
            c7 = bits[:, :, 0]
            for j in range(1, 7):
                c7 = c7 + (bits[:, :, j] << np.uint8(j))
            ct[..., 7] = c7
            fv[...] = ct.reshape(B, tc, F)
            fv -= 63.0
            fv *= os_
        else:
            np.multiply(oq, os_, out=fv)
        smax = os_.max()
        smin = os_.min()
        if not (np.isfinite(smax) and 0.0 < smin and smax < 0.1):
            ok = False
    lap("dequant")
    return final, ok


# revision 54
# speedup vs baseline: 1.1108x; 1.0479x over previous
"""Trainium2 Bass kernel for LocalDenseSynthesizerAttention.

Data-parallel over batch B=8 -> 8 cores, one batch each. The axon tunnel
(~45MB/s, effectively half-duplex) dominates, so the design minimizes wire
bytes (~21.8MB/call vs 43.4MB for the fp8/bf16 baseline; ~2.1x faster):
  - q shipped as packed 5-bit codes (8 codes -> 5 bytes) with per-row
    bf16-exact scales: the attention path is heavily damped (softmax over a
    45-wide window of tiny logits), so 5 bits cost only ~2e-3 of rel-err
  - v shipped int8 with per-64-block fp8 scales (pre-scaled x64 into fp8's
    normal range); v ships ONCE as a single zero-padded tensor shared by all
    chunk launches (window is local, pad=22), no per-chunk halo duplication
  - output quantized ON DEVICE to packed 7-bit codes (8 codes -> 7 bytes)
    with per-row f32 scales, unpacked/dequantized on host
  - all pack/unpack on device uses exact fp32 integer arithmetic (u8->f32
    convert, power-of-2 scaled floor via the +1.5*2^23 magic-add round), so
    device decode is bit-identical to the host's integer packing and
    independent of any convert-instruction rounding mode
  - projection weights shipped f32 ONCE (content-compared per call, reuses
    device-resident copies), AllGathered from 8-way shards on device, stored
    pre-transposed [128, KF, N] fp16 for the compute launches
  - device compute in fp16 (PE supports fp16 matmul): the extra mantissa
    bits vs bf16 pay for the int8/int5 wire budget; total measured rel-err
    1.63e-2 (gate 2e-2), bit-reproducible and matching the numpy simulation
  - codes+scales packed into one buffer per tensor (few device_puts; each
    put has a large fixed cost), puts dispatched from a small thread pool
  - compute split into 4 sequence chunks, one 8-core launch per chunk, so
    host quantization, uploads, exec, downloads and host dequant pipeline

The local window C=45 weighted sum is computed as banded matmuls: the banded
matrix B[s,t'] = attn[t0+t',h,s-t'] is an affine strided view of a zero-padded
attn tensor in DRAM, loaded matmul-ready via XBAR transpose-DMA.

Env knobs (defaults are the shipped config): KQBITS=5|6|8, KVBLK=1|0,
KOUT7=1|0, KCHUNKS=csv of chunk lengths, KTIMING=1 for per-phase timings.
NOTE: KQBITS=6 with KOUT7=1 miscompiles (a scheduling hazard corrupts the
last tile); both features are individually fine and the default q5+out7
combination is validated end-to-end.

Self-contained: hardcodes shapes from the problem spec.
"""
import sys
sys.path.insert(0, '/opt/trn_rl_repo')
import numpy as np

import concourse.bass as bass
import concourse.mybir as mybir
import concourse.tile as tile
from concourse import bacc
from concourse import masks

T, F = 2048, 512
H, C, DK = 8, 45, 64
HC = H * C          # 360
W = 128             # padded attn width per head (covers s-t' in [-63,127])
S = 64              # t' band-block size
PADV = 22           # (C-1)//2
KF = F // 128       # 4 contraction chunks
B = 8               # total batches / cores
FSH = F // B        # 64 weight-shard rows per core

VH = 64             # v halo rows each side (>= PADV, keeps tiles 128-aligned)
VOFF = VH - PADV    # chunk-vpad[r] = v_logical[r + VOFF]
CHUNKS = [(0, 512), (512, 512), (1024, 512), (1536, 512)]
import os as _os
if _os.environ.get("KCHUNKS"):
    _ls = [int(x) for x in _os.environ["KCHUNKS"].split(",")]
    assert sum(_ls) == T
    CHUNKS = []
    _c = 0
    for _l in _ls:
        CHUNKS.append((_c, _l))
        _c += _l

F16 = mybir.dt.float16
F32 = mybir.dt.float32
I8 = mybir.dt.int8
U8 = mybir.dt.uint8
FP8 = mybir.dt.float8e4
QD = 126.0          # int8 quant denominator (126 leaves headroom for the
                    # bf16 round-down of the scale: 126*1.002 < 126.5)
QD6 = 30.0          # 6-bit quant denominator (codes in [-30, 30])
QD5 = 15.0          # 5-bit quant denominator (codes in [-15, 15])
VDN = 118.0         # v block-quant denominator (fp8 scale round-down can
                    # inflate codes by up to 6.25%: 118*1.0625 < 127)
MAGIC = 12582912.0  # 1.5 * 2^23: fp32 add rounds the value to nearest int
QBITS = int(_os.environ.get("KQBITS", "5"))   # q wire precision: 5, 6, or 8
QROW = {5: 320, 6: 384, 8: 512}[QBITS]        # q bytes per row on the wire
VBLK = _os.environ.get("KVBLK", "1") == "1"   # v block-64 quant (else row)
NVS = 8 if VBLK else 4   # v scale bytes per row (8 fp8 / one f32)
OUT7 = _os.environ.get("KOUT7", "1") == "1"   # output packed 7-bit
# v ships as several tensors split at these rows (each 64 mod 128, so no
# halo-offset tile straddles a boundary); a chunk launch gates only on the
# parts it reads, so early chunks' downloads overlap later uploads
VBOUNDS = [0, 1088, T]
OROW = 448 if OUT7 else 512                   # output bytes per row
ODN = 62.0 if OUT7 else QD                    # output quant denominator

_CACHE = {}


# The build functions are compiled from a synthetic filename so the
# source-location debug info embedded in the BIR (and thus the NEFF
# cache key) does not depend on where this file lives.
_BUILD_SRC = r'''
def _build_w():
    """Weights launch (first call only): AllGather 8-way f32 weight shards,
    convert to fp16 in the matmul-ready [128, KF, N] layout, store to
    device-resident DRAM outputs."""
    nc = bacc.Bacc("TRN2", target_bir_lowering=False, debug=False,
                   num_devices=B, disable_frame_to_traceback=True)
    w1s = nc.dram_tensor("w1s", (FSH, F), F32, kind="ExternalInput")
    w2s = nc.dram_tensor("w2s", (FSH, HC), F32, kind="ExternalInput")
    w3s = nc.dram_tensor("w3s", (FSH, F), F32, kind="ExternalInput")
    wos = nc.dram_tensor("wos", (FSH, F), F32, kind="ExternalInput")
    w1f = nc.dram_tensor("w1f", (128, KF * F), F16, kind="ExternalOutput")
    w2f = nc.dram_tensor("w2f", (128, KF * HC), F16, kind="ExternalOutput")
    w3f = nc.dram_tensor("w3f", (128, KF * F), F16, kind="ExternalOutput")
    wof = nc.dram_tensor("wof", (128, KF * F), F16, kind="ExternalOutput")
    groups = [list(range(B))]
    with tile.TileContext(nc) as tc:
        with tc.tile_pool(name="dram", bufs=1, space="DRAM") as dp, \
             tc.tile_pool(name="sb", bufs=2) as sp:
            for idx, (shard, out, n) in enumerate(
                    ((w1s, w1f, F), (w2s, w2f, HC),
                     (w3s, w3f, F), (wos, wof, F))):
                stage = dp.tile([FSH, n], F32, name=f"st{idx}")
                full = dp.tile([F, n], F32, name=f"fu{idx}")
                # collectives cannot read IO tensors: stage shards first
                nc.sync.dma_start(stage[:, :], shard[:, :])
                nc.gpsimd.collective_compute(
                    "AllGather", mybir.AluOpType.bypass, groups,
                    [stage[:, :]], [full[:, :]])
                sb32 = sp.tile([128, KF, n], F32, tag=f"sb32_{n}")
                nc.sync.dma_start(
                    sb32[:], full[:, :].rearrange("(ko p) n -> p ko n", p=128))
                sb16 = sp.tile([128, KF, n], F16, tag=f"sb16_{n}")
                nc.scalar.copy(sb16[:], sb32[:])
                nc.sync.dma_start(
                    out[:, :].rearrange("p (ko n) -> p ko n", ko=KF), sb16[:])
    nc.compile()
    return nc


def _build_k(TC, C0):
    """Compute launch for one sequence chunk of TC rows starting at C0."""
    TCV = TC + 2 * VH           # v rows incl halo
    NT = TC // 128              # t-tiles in the chunk
    NTV = TCV // 128            # v tiles incl halo
    NB = TC // S                # band blocks
    nc = bacc.Bacc("TRN2", target_bir_lowering=False, debug=False,
                   num_devices=B, disable_frame_to_traceback=True)
    # qb = q rows (TC x QROW bytes: int8 codes, or 5/6-bit codes packed in
    # byte planes) then TC f32 row scales (raw bytes);
    # v parts (shared by all chunk launches), each codes-then-scales; a
    # chunk declares only the parts it reads, so its launch is not gated on
    # the other parts' uploads. Logical padded row r maps to v row r - VH;
    # the VH-row sequence-edge pads are zero-filled on device, not shipped.
    qb = nc.dram_tensor("qb", (TC * QROW + 4 * TC,), I8, kind="ExternalInput")
    vparts = []            # (tensor, row0, nrows) for the declared parts
    r_lo = max(C0 - VH, 0)                    # first v row this chunk reads
    r_hi = min(C0 + TC + VH, T)               # one past the last
    for pi in range(len(VBOUNDS) - 1):
        b0, b1 = VBOUNDS[pi], VBOUNDS[pi + 1]
        if r_lo < b1 and r_hi > b0:
            nn = b1 - b0
            vt = nc.dram_tensor(f"vp{pi}", (nn * F + NVS * nn,), I8,
                                kind="ExternalInput")
            vparts.append((vt, b0, nn))
    w1f = nc.dram_tensor("w1f", (128, KF * F), F16, kind="ExternalInput")
    w2f = nc.dram_tensor("w2f", (128, KF * HC), F16, kind="ExternalInput")
    w3f = nc.dram_tensor("w3f", (128, KF * F), F16, kind="ExternalInput")
    wof = nc.dram_tensor("wof", (128, KF * F), F16, kind="ExternalInput")
    oq = nc.dram_tensor("oq", (TC, OROW), I8, kind="ExternalOutput")
    os_ = nc.dram_tensor("os", (TC, 1), F32, kind="ExternalOutput")

    with tile.TileContext(nc) as tc:
        with tc.tile_pool(name="wpool", bufs=1) as wp, \
             tc.tile_pool(name="inpool", bufs=1) as inp, \
             tc.tile_pool(name="persist", bufs=1) as pers, \
             tc.tile_pool(name="work", bufs=2) as wk, \
             tc.tile_pool(name="band", bufs=4) as bp, \
             tc.tile_pool(name="psmain", bufs=2, space="PSUM") as psm, \
             tc.tile_pool(name="psband", bufs=4, space="PSUM") as psb, \
             tc.tile_pool(name="pstp", bufs=2, space="PSUM") as ptp, \
             tc.tile_pool(name="drampool", bufs=1, space="DRAM") as dp:

            # ---- weights to SBUF, [128, KF, n] fp16 (partition = contraction)
            w1_t = wp.tile([128, KF, F], F16, tag="w1")
            nc.sync.dma_start(
                w1_t[:], w1f[:, :].rearrange("p (ko n) -> p ko n", ko=KF))
            w2_t = wp.tile([128, KF, HC], F16, tag="w2")
            nc.sync.dma_start(
                w2_t[:], w2f[:, :].rearrange("p (ko n) -> p ko n", ko=KF))
            w3_t = wp.tile([128, KF, F], F16, tag="w3")
            nc.sync.dma_start(
                w3_t[:], w3f[:, :].rearrange("p (ko n) -> p ko n", ko=KF))
            wo_t = wp.tile([128, KF, F], F16, tag="wo")
            nc.sync.dma_start(
                wo_t[:], wof[:, :].rearrange("p (ko n) -> p ko n", ko=KF))

            ident = pers.tile([128, 128], F16, tag="ident")
            masks.make_identity(nc, ident[:])

            # ---- dequantize q and v (t-major int8 -> fp16), PE-transpose to
            # f-major [128 f, KF, t]
            qT = inp.tile([128, KF, TC], F16, tag="qT")
            vT = inp.tile([128, KF, TCV], F16, tag="vT")

            def _scale_ap(src_t, nrow, nbytes, tt):
                return src_t[nrow * nbytes + 4 * tt * 128:
                             nrow * nbytes + 4 * (tt + 1) * 128] \
                    .bitcast(F32).rearrange("(p n) -> p n", n=1)

            def _transpose_in(dst, dq, tt):
                for fo in range(KF):
                    pst = ptp.tile([128, 128], F16, tag="tp")
                    nc.tensor.transpose(
                        pst[:], dq[:, fo * 128:(fo + 1) * 128], ident[:])
                    nc.scalar.copy(dst[:, fo, tt * 128:(tt + 1) * 128],
                                   pst[:])

            A = mybir.AluOpType
            ts = nc.vector.tensor_scalar

            def _floor(out, in_, inv):
                # floor(in_/d) for small nonneg d-multiples-of-1/d: exact
                # round(in_*inv - 63/128) via the fp32 magic add
                ts(out, in_, inv, -0.4921875, A.mult, A.add)
                ts(out, out, MAGIC, -MAGIC, A.add, A.add)

            # ---- q tiles: unpack (if packed), dequant, transpose
            for tt in range(NT):
                sq = wk.tile([128, 1], F32, tag="sq")
                nc.sync.dma_start(sq[:], _scale_ap(qb, TC, QROW, tt))
                dq = wk.tile([128, F], F16, tag="dq")
                if QBITS == 8:
                    i8 = wk.tile([128, F], I8, tag="i8")
                    src = qb[tt * 128 * F:(tt + 1) * 128 * F]
                    nc.sync.dma_start(
                        i8[:], src.rearrange("(p n) -> p n", n=F))
                    nc.scalar.activation(dq[:], i8[:],
                                         mybir.ActivationFunctionType.Copy,
                                         scale=sq[:, :])
                    _transpose_in(qT, dq, tt)
                    continue
                # packed path: bytes hold a low field (6 or 5 bits) plus
                # high bits that assemble the remaining codes. All
                # arithmetic is exact fp32 on small integers.
                pk = wk.tile([128, QROW], U8, tag="pk")
                src = qb[tt * 128 * QROW:(tt + 1) * 128 * QROW].bitcast(U8)
                nc.sync.dma_start(
                    pk[:], src.rearrange("(p n) -> p n", n=QROW))
                pf = wk.tile([128, QROW], F32, tag="pf")
                nc.scalar.copy(pf[:], pk[:])
                hi = wk.tile([128, QROW], F32, tag="hi")
                lo = wk.tile([128, QROW], F32, tag="lo")
                ct = wk.tile([128, F], F32, tag="ct")
                if QBITS == 6:
                    # planes P_j (128 cols): low 6 = code of f=4k+j; high 2
                    # = bits [2j,2j+2) of the f=4k+3 code
                    _floor(hi[:], pf[:], 1.0 / 64)
                    ts(lo[:], hi[:], -64.0, None, A.mult)
                    nc.vector.tensor_add(out=lo[:], in0=lo[:], in1=pf[:])
                    c4 = ct[:].rearrange("p (k j) -> p k j", j=4)
                    for j in range(3):
                        nc.vector.tensor_copy(
                            out=c4[:, :, j], in_=lo[:, j * 128:(j + 1) * 128])
                    t1 = wk.tile([128, 128], F32, tag="t1")
                    ts(t1[:], hi[:, 128:256], 4.0, None, A.mult)
                    nc.vector.tensor_add(out=t1[:], in0=t1[:],
                                         in1=hi[:, 0:128])
                    t2 = wk.tile([128, 128], F32, tag="t2")
                    ts(t2[:], hi[:, 256:384], 16.0, None, A.mult)
                    nc.vector.tensor_add(out=c4[:, :, 3], in0=t1[:],
                                         in1=t2[:])
                    qbias = -32.0
                else:
                    # QBITS == 5: planes P_j (j<5, 64 cols): low 5 = code of
                    # f=8k+j; high 3 = bits [3j,3j+3) of G = c5|c6<<5|c7<<10
                    _floor(hi[:], pf[:], 1.0 / 32)
                    ts(lo[:], hi[:], -32.0, None, A.mult)
                    nc.vector.tensor_add(out=lo[:], in0=lo[:], in1=pf[:])
                    c8 = ct[:].rearrange("p (k j) -> p k j", j=8)
                    for j in range(5):
                        nc.vector.tensor_copy(
                            out=c8[:, :, j], in_=lo[:, j * 64:(j + 1) * 64])
                    g = wk.tile([128, 64], F32, tag="g")
                    t1 = wk.tile([128, 64], F32, tag="t1")
                    ts(g[:], hi[:, 64:128], 8.0, None, A.mult)
                    nc.vector.tensor_add(out=g[:], in0=g[:], in1=hi[:, 0:64])
                    for j, w in ((2, 64.0), (3, 512.0), (4, 4096.0)):
                        ts(t1[:], hi[:, j * 64:(j + 1) * 64], w, None, A.mult)
                        nc.vector.tensor_add(out=g[:], in0=g[:], in1=t1[:])
                    f1 = wk.tile([128, 64], F32, tag="f1")
                    _floor(f1[:], g[:], 1.0 / 32)
                    ts(t1[:], f1[:], -32.0, None, A.mult)
                    nc.vector.tensor_add(out=c8[:, :, 5], in0=t1[:],
                                         in1=g[:])
                    f2 = wk.tile([128, 64], F32, tag="f2")
                    _floor(f2[:], f1[:], 1.0 / 32)
                    ts(t1[:], f2[:], -32.0, None, A.mult)
                    nc.vector.tensor_add(out=c8[:, :, 6], in0=t1[:],
                                         in1=f1[:])
                    nc.vector.tensor_copy(out=c8[:, :, 7], in_=f2[:])
                    qbias = -16.0
                # dequant: (c + qbias) * s = c*s + qbias*s
                nbias = wk.tile([128, 1], F32, tag="nbias")
                ts(nbias[:], sq[:], qbias, None, A.mult)
                nc.scalar.activation(dq[:], ct[:],
                                     mybir.ActivationFunctionType.Identity,
                                     bias=nbias[:, :], scale=sq[:, :])
                _transpose_in(qT, dq, tt)

            # ---- v tiles: int8 dequant, transpose. Tile tt covers padded
            # rows [C0+128tt, C0+128(tt+1)) = vb rows shifted by -VH; tiles
            # overlapping the sequence edge are zero-filled then partially
            # loaded.
            for tt in range(NTV):
                p0 = C0 + tt * 128          # first padded row of the tile
                a = max(p0, VH) - p0        # first valid partition
                b = min(p0 + 128, VH + T) - p0   # one past last valid
                r0 = p0 - VH + a            # first v row
                nrow = b - a
                vt, vr0, vnr = next((t, o, nn) for t, o, nn in vparts
                                    if o <= r0 < o + nn)
                assert r0 + nrow <= vr0 + vnr   # aligned split: no straddle
                rr = r0 - vr0
                i8 = wk.tile([128, F], I8, tag="i8")
                if nrow < 128:
                    nc.any.memzero(i8[:])
                src = vt[rr * F:(rr + nrow) * F]
                nc.sync.dma_start(
                    i8[a:b, :], src.rearrange("(p n) -> p n", n=F))
                dq = wk.tile([128, F], F16, tag="dq")
                if VBLK:
                    # 8 fp8 block scales per row, pre-multiplied by 64 on
                    # the host so they sit in fp8's normal range
                    s8 = wk.tile([128, 8], FP8, tag="s8")
                    if nrow < 128:
                        # fp8 1.0 in the pad rows (codes there are 0; any
                        # finite scale works, garbage could decode to NaN)
                        nc.any.memset(s8[:], 1.0)
                    ssrc = vt[vnr * F + 8 * rr:vnr * F + 8 * (rr + nrow)] \
                        .bitcast(FP8).rearrange("(p n) -> p n", n=8)
                    nc.sync.dma_start(s8[a:b, :], ssrc)
                    sf = wk.tile([128, 8], F32, tag="sf")
                    nc.scalar.copy(sf[:], s8[:])
                    ts(sf[:], sf[:], 1.0 / 64, None, A.mult)
                    vf = wk.tile([128, F], F32, tag="vf")
                    nc.scalar.copy(vf[:], i8[:])
                    nc.vector.tensor_mul(
                        out=dq[:].rearrange("p (g k) -> p g k", k=64),
                        in0=vf[:].rearrange("p (g k) -> p g k", k=64),
                        in1=sf[:, :, None].to_broadcast((128, 8, 64)))
                else:
                    sq = wk.tile([128, 1], F32, tag="sq")
                    if nrow < 128:
                        nc.any.memset(sq[:], 1.0)
                    ssrc = vt[vnr * F + 4 * rr:vnr * F + 4 * (rr + nrow)] \
                        .bitcast(F32).rearrange("(p n) -> p n", n=1)
                    nc.sync.dma_start(sq[a:b, :], ssrc)
                    nc.scalar.activation(dq[:], i8[:],
                                         mybir.ActivationFunctionType.Copy,
                                         scale=sq[:, :])
                _transpose_in(vT, dq, tt)

            # ---- DRAM scratch
            # vproj rows j = w3-projection of v_in row j; v rows outside the
            # sequence are int8 zeros (scale 1) and project to exact zeros
            vproj = dp.tile([TCV, F], F16)
            # apad: 1 guard row + TC data rows + 1 guard row, row = [8 x 128]
            apad = dp.tile([TC + 2, H * W], F16)

            # zero tile for apad guards
            z_t = pers.tile([128, H * W], F16, tag="zt")
            nc.any.memzero(z_t[:])
            nc.sync.dma_start(apad[0:1, :], z_t[0:1, :])
            nc.sync.dma_start(apad[TC + 1:TC + 2, :], z_t[0:1, :])

            # ---- persistent SBUF activations
            qrT = pers.tile([128, KF, TC], F16, tag="qrT")  # relu(q@w1) f-major
            xT = pers.tile([128, KF, TC], F16, tag="xT")    # band out, f-major

            # ================= Phase A: q-proj + relu (f-major out) ===========
            for fo in range(KF):
                for b0 in range(0, TC, 512):
                    bw = min(512, TC - b0)
                    ps = psm.tile([128, 512], F32, tag="mm")
                    for k in range(KF):
                        nc.tensor.matmul(
                            ps[:, 0:bw], w1_t[:, k, fo * 128:(fo + 1) * 128],
                            qT[:, k, b0:b0 + bw],
                            start=(k == 0), stop=(k == KF - 1))
                    nc.scalar.activation(qrT[:, fo, b0:b0 + bw],
                                         ps[:, 0:bw],
                                         mybir.ActivationFunctionType.Relu)

            # ================= Phase C: v-proj (t-major out) -> vproj =========
            for tb in range(NTV):
                ps = psm.tile([128, 512], F32, tag="mm")
                for k in range(KF):
                    nc.tensor.matmul(
                        ps[:], vT[:, k, tb * 128:(tb + 1) * 128],
                        w3_t[:, k, :],
                        start=(k == 0), stop=(k == KF - 1))
                v_sb = wk.tile([128, F], F16, tag="vsb")
                nc.scalar.copy(v_sb[:], ps[:])
                nc.sync.dma_start(vproj[tb * 128:(tb + 1) * 128, :], v_sb[:])

            # ====== Phase B: s-proj (t-major) + softmax -> apad (padded) ======
            for tb in range(NT):
                ps = psm.tile([128, 512], F32, tag="mm")
                for k in range(KF):
                    nc.tensor.matmul(
                        ps[:, 0:HC], qrT[:, k, tb * 128:(tb + 1) * 128],
                        w2_t[:, k, :],
                        start=(k == 0), stop=(k == KF - 1))
                e_t = wk.tile([128, HC], F32, tag="et")
                nc.scalar.activation(e_t[:], ps[:, 0:HC],
                                     mybir.ActivationFunctionType.Exp)
                zs = wk.tile([128, H], F32, tag="zs")
                nc.vector.reduce_sum(zs[:],
                                     e_t[:].rearrange("p (h c) -> p h c", c=C),
                                     axis=mybir.AxisListType.X)
                rz = wk.tile([128, H], F32, tag="rz")
                nc.vector.reciprocal(rz[:], zs[:])
                ap_t = wk.tile([128, H * W], F16, tag="apad")
                if tb < 2:
                    # zero the pad region once per pool slot (bufs=2); the pad
                    # columns are never overwritten afterwards
                    nc.any.memzero(ap_t[:])
                nc.vector.tensor_mul(
                    out=ap_t[:].rearrange("p (h w) -> p h w", w=W)[:, :, 0:C],
                    in0=e_t[:].rearrange("p (h c) -> p h c", c=C),
                    in1=rz[:, :, None].to_broadcast((128, H, C)))
                nc.sync.dma_start(apad[1 + tb * 128:1 + (tb + 1) * 128, :],
                                  ap_t[:])

            # ================= Phase D: banded attention matmuls ==============
            # x[t', h*64+d] = sum_s vproj[VOFF+t0+s, h*64+d] * B_h[s, t']
            # B_h loaded via transpose-DMA of sheared apad view.
            apad_h = apad.tensor  # underlying DRAM handle
            apad_off = apad.offset if isinstance(apad.offset, int) else 0
            for g in range(NB // 4):    # groups of 4 band blocks = 256 t'
                pss = [psb.tile([128, 512], F32, tag="px", name=f"px{g}_{pi}")
                       for pi in range(4)]
                for j in range(4):
                    bi = g * 4 + j
                    t0 = S * bi
                    vsp = wk.tile([128, F], F16, tag="vsp")
                    nc.sync.dma_start(vsp[:],
                                      vproj[VOFF + t0:VOFF + t0 + 128, :])
                    for p in range(4):      # head pairs
                        for i in range(2):
                            h = 2 * p + i
                            b_t = bp.tile([W, S], F16, tag="bt")
                            src = bass.AP(
                                tensor=apad_h,
                                offset=apad_off + (1 + t0) * (H * W) + h * W,
                                ap=[[H * W - 1, S], [1, W]])
                            eng = nc.scalar if h % 2 else nc.sync
                            eng.dma_start_transpose(b_t[:], src)
                            # lhsT = v head-pair [128, 128]; valid out rows are
                            # [i*64:(i+1)*64]; the other half is garbage and
                            # ignored at copyback.
                            nc.tensor.matmul(
                                pss[p][:, j * 128 + i * 64:
                                       j * 128 + (i + 1) * 64],
                                vsp[:, p * 128:(p + 1) * 128], b_t[:],
                                start=True, stop=True)
                # copy valid quadrants -> xT (f-major): fold p rows 0:63 = head
                # 2p (cols i=0), rows 64:127 = head 2p+1 (cols i=1)
                for p in range(4):
                    ps3 = pss[p][:].rearrange("d (j i k) -> d j i k", j=4, i=2)
                    dst = xT[:, p, g * 256:(g + 1) * 256] \
                        .rearrange("d (j k) -> d j k", j=4)
                    nc.vector.tensor_copy(out=dst[0:64], in_=ps3[0:64, :, 0, :])
                    nc.vector.tensor_copy(out=dst[64:128],
                                          in_=ps3[64:128, :, 1, :])

            # ========= Phase E: out-proj + per-row int8 quantization ==========
            for tb in range(NT):
                ps = psm.tile([128, 512], F32, tag="mm")
                for k in range(KF):
                    nc.tensor.matmul(
                        ps[:], xT[:, k, tb * 128:(tb + 1) * 128],
                        wo_t[:, k, :],
                        start=(k == 0), stop=(k == KF - 1))
                am = wk.tile([128, 1], F32, tag="am")
                nc.vector.reduce_max(am[:], ps[:], axis=mybir.AxisListType.X,
                                     apply_absolute_value=True)
                rz = wk.tile([128, 1], F32, tag="orz")
                nc.vector.reciprocal(rz[:], am[:])
                rs = wk.tile([128, 1], F32, tag="ors")
                nc.vector.tensor_scalar_mul(rs[:], rz[:], ODN)
                y = wk.tile([128, F], F32, tag="oy")
                nc.scalar.activation(y[:], ps[:],
                                     mybir.ActivationFunctionType.Copy,
                                     scale=rs[:, :])
                ost = wk.tile([128, 1], F32, tag="ost")
                nc.vector.tensor_scalar_mul(ost[:], am[:], 1.0 / ODN)
                nc.scalar.dma_start(os_[tb * 128:(tb + 1) * 128, :], ost[:])
                if not OUT7:
                    # round to nearest int (RNE) via magic add/sub, then
                    # convert: the value is exactly integral so the
                    # convert's rounding mode is irrelevant
                    yr = wk.tile([128, F], F32, tag="oyr")
                    ts(yr[:], y[:], MAGIC, -MAGIC, A.add, A.add)
                    oqt = wk.tile([128, F], I8, tag="oqt")
                    nc.vector.tensor_copy(out=oqt[:], in_=yr[:])
                    nc.sync.dma_start(oq[tb * 128:(tb + 1) * 128, :], oqt[:])
                    continue
                # biased 7-bit codes in [1,125]: rint via magic, +63 folded
                # into the second (still exact-integer) add
                yr = wk.tile([128, F], F32, tag="oyr")
                ts(yr[:], y[:], MAGIC, 63.0 - MAGIC, A.add, A.add)
                # pack 8 codes -> 7 bytes: B_j = c_j + 128*bit_j(c7)
                c8 = yr[:].rearrange("p (k j) -> p k j", j=8)
                pko = wk.tile([128, 7 * 64], F32, tag="pko")
                b0 = wk.tile([128, 64], F32, tag="pb0")
                b1 = wk.tile([128, 64], F32, tag="pb1")
                tbit = wk.tile([128, 64], F32, tag="tbit")
                nc.vector.tensor_copy(out=b0[:], in_=c8[:, :, 7])
                bs, bd = b0, b1
                for j in range(7):
                    # bd = floor(bs/2); fractions are {0,.5} so -0.25 rounds
                    ts(bd[:], bs[:], 0.5, -0.25, A.mult, A.add)
                    ts(bd[:], bd[:], MAGIC, -MAGIC, A.add, A.add)
                    ts(tbit[:], bd[:], -2.0, None, A.mult)
                    nc.vector.tensor_add(out=tbit[:], in0=tbit[:],
                                         in1=bs[:])
                    ts(tbit[:], tbit[:], 128.0, None, A.mult)
                    nc.vector.tensor_add(out=pko[:, j * 64:(j + 1) * 64],
                                         in0=tbit[:], in1=c8[:, :, j])
                    bs, bd = bd, bs
                oqt = wk.tile([128, 7 * 64], U8, tag="oqt7")
                nc.vector.tensor_copy(out=oqt[:], in_=pko[:])
                nc.sync.dma_start(
                    oq[tb * 128:(tb + 1) * 128, :].bitcast(U8), oqt[:])

    nc.compile()
    return nc
'''

exec(compile(_BUILD_SRC, "bass_build_k", "exec"), globals())


def _make_exec(nc, devices):
    """Cached jitted executable for one bass module; outputs come from
    donated on-device zero buffers (mkzeros)."""
    import jax
    import jax.numpy as jnp
    from jax.sharding import Mesh, PartitionSpec, NamedSharding
    from jax.experimental.shard_map import shard_map
    from concourse.bass2jax import _bass_exec_p, partition_id_tensor

    partition_name = (nc.partition_id_tensor.name
                      if nc.partition_id_tensor else None)
    in_names, out_names, out_avals = [], [], []
    for alloc in nc.m.functions[0].allocations:
        if not isinstance(alloc, mybir.MemoryLocationSet):
            continue
        if alloc.kind not in ("ExternalInput", "ExternalOutput"):
            continue
        name = alloc.memorylocations[0].name
        if alloc.kind == "ExternalInput":
            if name != partition_name:
                in_names.append(name)
        else:
            out_avals.append(jax.core.ShapedArray(
                tuple(alloc.tensor_shape), mybir.dt.np(alloc.dtype)))
            out_names.append(name)
    n_params, n_outs = len(in_names), len(out_avals)
    in_names_all = list(in_names) + list(out_names)
    if partition_name is not None:
        in_names_all.append(partition_name)

    def _body(*args):
        operands = list(args)
        if partition_name is not None:
            operands.append(partition_id_tensor())
        return tuple(_bass_exec_p.bind(
            *operands,
            out_avals=tuple(out_avals),
            in_names=tuple(in_names_all),
            out_names=tuple(out_names),
            lowering_input_output_aliases=(),
            sim_require_finite=True,
            sim_require_nnan=True,
            nc=nc))

    n = len(devices)
    mesh = Mesh(np.asarray(devices), ("core",))
    shard = NamedSharding(mesh, PartitionSpec("core"))
    n_args = n_params + n_outs
    donate = tuple(range(n_params, n_args))
    mkzeros = jax.jit(
        lambda: tuple(jnp.zeros((n * a.shape[0], *a.shape[1:]), a.dtype)
                      for a in out_avals),
        out_shardings=(shard,) * n_outs)
    in_specs = (PartitionSpec("core"),) * n_args
    out_specs = (PartitionSpec("core"),) * n_outs
    sharded = jax.jit(
        shard_map(_body, mesh=mesh, in_specs=in_specs, out_specs=out_specs,
                  check_rep=False),
        donate_argnums=donate, keep_unused=True)
    return {"sharded": sharded, "mkzeros": mkzeros, "in_names": in_names,
            "out_names": out_names, "shard": shard, "n": n}


def _get_state():
    if "state" in _CACHE:
        return _CACHE["state"]
    import jax
    from concurrent.futures import ThreadPoolExecutor
    from concourse.bass2jax import install_neuronx_cc_hook
    install_neuronx_cc_hook()
    devices = jax.devices()[:B]
    wexec = _make_exec(_build_w(), devices)
    kexecs = {}
    for c0, tc in CHUNKS:
        kexecs[(tc, c0)] = _make_exec(_build_k(tc, c0), devices)
    state = {"w": wexec, "k": kexecs,
             "pool": ThreadPoolExecutor(max_workers=3)}
    _CACHE["state"] = state
    return state


def _to_bf16_f32(x32):
    """fp32 -> bf16 via round-half-up on the upper 16 bits, returned as
    exact f32 values (so host and device share bit-identical scales)."""
    x32 = np.ascontiguousarray(x32, np.float32)
    tmp = x32.view(np.uint32) + np.uint32(0x8000)
    np.bitwise_and(tmp, np.uint32(0xFFFF0000), out=tmp)
    return tmp.view(np.float32)


def _quant_into(x, codes, scales, ybuf):
    """Per-row symmetric int8 quantization of (B, n, F) fp32 into
    preallocated codes (int8) and scales (f32, bf16-exact) views."""
    n = x.shape[1]
    a = np.maximum(x.max(axis=-1), -x.min(axis=-1))
    np.maximum(a, 1e-30, out=a)
    s = _to_bf16_f32(a / QD)
    y = ybuf[:, :n]
    np.multiply(x, (1.0 / s)[..., None], out=y)
    np.rint(y, out=y)
    codes[...] = y          # values are exactly integral: cast is exact
    scales[...] = s


def _quant6_into(x, codes, scales, ybuf):
    """Per-row 6-bit quantization of (B, n, F) fp32, packed 4 codes -> 3
    bytes in the plane layout the device kernel unpacks."""
    n = x.shape[1]
    a = np.maximum(x.max(axis=-1), -x.min(axis=-1))
    np.maximum(a, 1e-30, out=a)
    s = _to_bf16_f32(a / QD6)
    y = ybuf[:, :n]
    np.multiply(x, (1.0 / s)[..., None], out=y)
    np.rint(y, out=y)
    y += 32.0                   # biased codes in [2, 62]
    c = y.astype(np.uint8).reshape(-1, n, 128, 4)
    b3 = c[..., 3]
    cu = codes.view(np.uint8)
    cu[:, :, 0:128] = c[..., 0] + ((b3 & 3) << 6)
    cu[:, :, 128:256] = c[..., 1] + (((b3 >> 2) & 3) << 6)
    cu[:, :, 256:384] = c[..., 2] + ((b3 >> 4) << 6)
    scales[...] = s


def _quant5_into(x, codes, scales, ybuf):
    """Per-row 5-bit quantization of (B, n, F) fp32, packed 8 codes -> 5
    bytes in the plane layout the device kernel unpacks."""
    n = x.shape[1]
    a = np.maximum(x.max(axis=-1), -x.min(axis=-1))
    np.maximum(a, 1e-30, out=a)
    s = _to_bf16_f32(a / QD5)
    y = ybuf[:, :n]
    np.multiply(x, (1.0 / s)[..., None], out=y)
    np.rint(y, out=y)
    y += 16.0                   # biased codes in [1, 31]
    c = y.astype(np.uint8).reshape(-1, n, 64, 8)
    G = (c[..., 5].astype(np.uint16) + (c[..., 6].astype(np.uint16) << 5)
         + (c[..., 7].astype(np.uint16) << 10))
    cu = codes.view(np.uint8)
    for j in range(5):
        cu[:, :, j * 64:(j + 1) * 64] = \
            c[..., j] + (((G >> (3 * j)) & 7).astype(np.uint8) << 5)
    scales[...] = s


def _quant_vblk_into(x, codes, scales_u8, ybuf):
    """Block-64 int8 quantization of (B, n, F) fp32; 8 fp8 scales per row,
    shipped pre-multiplied by 64 (fp8 normal range), dequant divides."""
    import ml_dtypes
    n = x.shape[1]
    xb = x.reshape(-1, n, F // 64, 64)
    a = np.maximum(xb.max(axis=-1), -xb.min(axis=-1))
    np.maximum(a, 0.25, out=a)
    s8 = (a * (64.0 / VDN)).astype(ml_dtypes.float8_e4m3)
    s = s8.astype(np.float32) * (1.0 / 64)
    y = ybuf[:, :n].reshape(-1, n, F // 64, 64)
    np.multiply(xb, (1.0 / s)[..., None], out=y)
    np.rint(y, out=y)
    codes.reshape(-1, n, F // 64, 64)[...] = y
    scales_u8[...] = s8.view(np.uint8)


def _weights_device(st, w1, w2, w3, w_out):
    """Device-resident fp16 weights, re-uploaded only when contents change."""
    import jax
    ws = [np.ascontiguousarray(np.asarray(w), np.float32)
          for w in (w1, w2, w3, w_out)]
    cached = _CACHE.get("wfull")
    if cached is not None and all(
            np.array_equal(a, b) for a, b in zip(ws, _CACHE["whost"])):
        return cached
    wx = st["w"]
    wzeros = wx["mkzeros"]()
    arrs = {"w1s": ws[0], "w2s": ws[1], "w3s": ws[2], "wos": ws[3]}
    wouts = wx["sharded"](*[arrs[n] for n in wx["in_names"]], *wzeros)
    wfull = dict(zip(wx["out_names"], wouts))
    _CACHE["whost"] = ws
    _CACHE["wfull"] = wfull
    return wfull


def kernel(query, key, value, w1, w2, w3, w_out, _trace=False):
    out, ok = _kernel_once(query, key, value, w1, w2, w3, w_out)
    # The remote runtime very occasionally returns a stale/corrupt buffer.
    # Output row scales from a real run are all in (0, ~2e-3); a stale
    # (zero-initialized or garbage) buffer fails this. Retry once.
    if not ok:
        out, ok = _kernel_once(query, key, value, w1, w2, w3, w_out)
    return out


def _kernel_once(query, key, value, w1, w2, w3, w_out):
    import jax, os, time
    st = _get_state()
    put = jax.device_put
    pool = st["pool"]
    timing = os.environ.get("KTIMING")
    tt0 = time.perf_counter()
    lap = lambda tag: timing and print(
        f"  [{tag}] {time.perf_counter() - tt0:.3f}s", flush=True)

    query = np.asarray(query)
    value = np.asarray(value)

    wfull = _weights_device(st, w1, w2, w3, w_out)
    # scratch fp32 buffer shared by all quantizations (v uses all T rows)
    ybuf = _CACHE.get("ybuf")
    if ybuf is None:
        ybuf = _CACHE["ybuf"] = np.empty((B, T, F), np.float32)
    lap("weights")

    def _vpart(row0, nrows):
        blob = np.empty((B, nrows * F + NVS * nrows), np.int8)
        codes = blob[:, :nrows * F].reshape(B, nrows, F)
        vsl = value[:, row0:row0 + nrows]
        if VBLK:
            scales = blob[:, nrows * F:].view(np.uint8).reshape(B, nrows, 8)
            _quant_vblk_into(vsl, codes, scales, ybuf)
        else:
            scales = blob[:, nrows * F:].view(np.float32)
            _quant_into(vsl, codes, scales, ybuf)
        return pool.submit(put, blob.reshape(-1), shard)

    pending = []
    vfuts = {}
    for i, (c0, tc) in enumerate(CHUNKS):
        kx = st["k"][(tc, c0)]
        shard = kx["shard"]
        zeros_f = pool.submit(kx["mkzeros"])
        # q blob: codes then f32 row scales as raw bytes; ship it while
        # v is still being quantized
        qblob = np.empty((B, tc * QROW + 4 * tc), np.int8)
        qcodes = qblob[:, :tc * QROW].reshape(B, tc, QROW)
        qscales = qblob[:, tc * QROW:].view(np.float32)
        qfun = {5: _quant5_into, 6: _quant6_into, 8: _quant_into}[QBITS]
        qfun(query[:, c0:c0 + tc], qcodes, qscales, ybuf)
        qb_f = pool.submit(put, qblob.reshape(-1), shard)
        lap(f"qput{i}")
        # ship each v part just before the first launch that needs it
        for name in kx["in_names"]:
            if name.startswith("vp") and name not in vfuts:
                pi = int(name[2:])
                vfuts[name] = _vpart(VBOUNDS[pi], VBOUNDS[pi + 1] - VBOUNDS[pi])
                lap(f"{name}put")
        arrays = {"qb": qb_f.result(),
                  "w1f": wfull["w1f"], "w2f": wfull["w2f"],
                  "w3f": wfull["w3f"], "wof": wfull["wof"]}
        for name, fut in vfuts.items():
            arrays[name] = fut.result()
        ins = [arrays[n] for n in kx["in_names"]]
        outs = kx["sharded"](*ins, *zeros_f.result())
        for o_ in outs:
            o_.copy_to_host_async()   # start D2H as soon as exec finishes
        lap(f"launch{i}")
        pending.append(dict(zip(kx["out_names"], outs)))

    # ---- collect: unpack/dequantize rows with their f32 scales
    final = np.empty((B, T, F), np.float32)
    ok = True
    for ci, ((c0, tc), outs) in enumerate(zip(CHUNKS, pending)):
        oq = np.asarray(outs["oq"]).reshape(B, tc, OROW)
        os_ = np.asarray(outs["os"]).reshape(B, tc, 1)
        lap(f"fetch{ci}")
        fv = final[:, c0:c0 + tc]
        if OUT7:
            raw = oq.view(np.uint8).reshape(B, tc, 7, 64)
            low = raw & np.uint8(127)
            bits = raw >> np.uint8(7)
            ct = np.empty((B, tc, 64, 8), np.uint8)
            for j in range(7):
                ct[..., j] = low[:, :, j]
            c7 = bits[:, :, 0]
            for j in range(1, 7):
                c7 = c7 + (bits[:, :, j] << np.uint8(j))
            ct[..., 7] = c7
            fv[...] = ct.reshape(B, tc, F)
            fv -= 63.0
            fv *= os_
        else:
            np.multiply(oq, os_, out=fv)
        smax = os_.max()
        smin = os_.min()
        if not (np.isfinite(smax) and 0.0 < smin and smax < 0.1):
            ok = False
    lap("dequant")
    return final, ok
